# revision 2
# baseline (speedup 1.0000x reference)
"""Bass/Trainium2 kernel for nn_GRUClassifier: 2-layer BiGRU + max-pool + MLP head.

v2 sharding: 8 cores x 8 sequences, no duplicated compute, no cross-core
exchange. Each core runs two independent 8-lane recurrence chains per phase
(fwd + bwd direction), interleaved so each chain's serial step latency hides
behind the other's. Phase 1 = L0 both directions; phase 2 = L1 both
directions with on-the-fly max pool; W1-half head GEMM on device.

Latency cuts vs v1: xp contributions for the r,z gates are accumulated into
PSUM by an identity-matmul (removes 2 DVE ops/step), the b_hh n-gate bias is
added via a K=1 ones-row matmul (removes the 4-way u-loop), and bulk DMAs are
issued from the GpSimd queue (SP sequencer was 2.4us/DMA).

All matmul operands fp16, accumulation fp32 in PSUM. Backward-direction
sequence reversal is host-side for L0 inputs (reversed token stream), output
position alignment for the L0 bwd chain is compile-time reversed write
offsets, and the L1 bwd chain reads its (position-ordered) xp stream at
compile-time reversed offsets.
"""
import os
import sys
import numpy as np

sys.path.insert(0, "/opt/trn_rl_repo")

B, T, E, H, V = 64, 256, 300, 512, 50000
EP = 384            # E padded to 3*128
G = 3 * H           # 1536 gate rows = 12 chunks of 128
BL = 8              # batch per core
NTOK = T * BL       # 2048
SBLK = 16           # steps per xpt/y block
RB = SBLK * BL      # 128 cols per recurrence block
NRB = T // SBLK     # 16 recurrence blocks
GBC = 256           # gemm block cols
NGB = NTOK // GBC   # 8 gemm blocks
GSTEP = GBC // BL   # 32 steps per gemm block

_CACHE = {}


def _patch_drain():
    """walrus CoreV3 rejects CTRL (Drain) instructions with too many sem
    waits; split the tail-drain's waits across preceding sync nops."""
    from concourse import mybir
    from concourse.tile import TileContext
    from concourse.vector_clock import ScopedClock

    if getattr(TileContext, "_drain_patched", False):
        return
    MAXW = 1

    def _drain_and_barrier(self, tick_clock, wait_clock):
        drain_inst = self.nc.sync.drain()
        wait_clock.add_sem_waits(
            drain_inst.ins, ScopedClock({None: tick_clock.global_clock})
        )
        si = drain_inst.ins.sync_info
        if si is not None and si.on_wait and len(si.on_wait) > MAXW:
            waits = list(si.on_wait)
            si.on_wait = waits[:MAXW]
            for i in range(MAXW, len(waits), MAXW):
                nop = self.nc.sync.nop(nofuse=True, hint="drain_wait_split")
                nsi = nop.ins.sync_info
                if nsi is None:
                    nop.ins.sync_info = mybir.SyncInfo(
                        on_wait=waits[i : i + MAXW], on_update=[]
                    )
                else:
                    nsi.on_wait = waits[i : i + MAXW]
        self.nc.all_engine_barrier()
        assert self.sems is not None
        popped = self.nc._tile_sem_poison_stack.pop()
        assert popped is self._sem_poison
        self.nc.clear_and_free_semaphores(list(self.sems.allocated().values()))
        self.nc.all_engine_barrier()

    TileContext._drain_and_barrier = _drain_and_barrier
    TileContext._drain_patched = True


def _split_multiwaits(nc, mybir, maxw=1):
    """walrus CoreV2/V3 setupSyncWait rejects instructions with more than one
    sem wait; split extras onto preceding same-engine nops."""
    cnt = 0
    for fn in nc.m.functions:
        for bb in fn.blocks:
            insts = bb.instructions
            out = []
            changed = False
            for inst in insts:
                si = getattr(inst, "sync_info", None)
                eng = getattr(inst, "engine", None)
                if (
                    si is not None
                    and si.on_wait
                    and len(si.on_wait) > maxw
                    and eng is not None
                    and eng != mybir.EngineType.Unassigned
                ):
                    waits = list(si.on_wait)
                    for w in waits[:-maxw]:
                        nop = mybir.InstNoOp(
                            name=f"ws_nop_{cnt}", ins=[], outs=[]
                        )
                        cnt += 1
                        nop.engine = eng
                        nop.sync_info = mybir.SyncInfo(
                            on_wait=[w], on_update=[]
                        )
                        out.append(nop)
                    si.on_wait = waits[-maxw:]
                    changed = True
                out.append(inst)
            if changed:
                bb.instructions = out
    return cnt


def _build_nc():
    from concourse import bass, mybir
    from concourse.tile import TileContext

    _patch_drain()
    f16 = mybir.dt.float16
    f32 = mybir.dt.float32
    AF = mybir.ActivationFunctionType
    OP = mybir.AluOpType

    nc = bass.Bass(target_bir_lowering=False)

    def par(name, shape, dt=f16, out=False):
        return nc.declare_dram_parameter(name, list(shape), dt, isOutput=out)

    eTf = par("eTf", [128, 3, NTOK])          # fwd-order embedded input
    eTb = par("eTb", [128, 3, NTOK])          # reversed-order
    wihf = par("wihf", [128, 3, G])           # L0 W_ih^T k-tiles
    wihb = par("wihb", [128, 3, G])
    whhf = par("whhf", [128, 4, G])           # L0 W_hh^T k-tiles
    whhb = par("whhb", [128, 4, G])
    wLf = par("wLf", [128, 8, G])             # L1 W_ih^T k-tiles (yF then yB)
    wLb = par("wLb", [128, 8, G])
    whhLf = par("whhLf", [128, 4, G])
    whhLb = par("whhLb", [128, 4, G])
    biasf = par("biasf", [128, 12], f32)      # xp bias per gate chunk col
    biasb = par("biasb", [128, 12], f32)
    biasLf = par("biasLf", [128, 12], f32)
    biasLb = par("biasLb", [128, 12], f32)
    nbias = par("nbias", [1, 4, 512])         # b_hh n-gate rows: L0f,L0b,L1f,L1b
    id128 = par("id128", [128, 128])
    w1f = par("w1f", [128, 4, 128])           # classifier W1^T k-tiles, f half
    w1b = par("w1b", [128, 4, 128])
    headout = par("headout", [128, BL], f32, out=True)

    xp0f = nc.dram_tensor("xp0f", [128, 12, NTOK], f16)
    xp0b = nc.dram_tensor("xp0b", [128, 12, NTOK], f16)
    xpLf = nc.dram_tensor("xpLf", [128, 12, NTOK], f16)
    xpLb = nc.dram_tensor("xpLb", [128, 12, NTOK], f16)
    yF = nc.dram_tensor("yF", [128, 4, NTOK], f16)
    yB = nc.dram_tensor("yB", [128, 4, NTOK], f16)

    with TileContext(nc) as tc:
        with (
            tc.tile_pool(name="wpool", bufs=1) as wp,
            tc.tile_pool(name="io", bufs=3) as io,
            tc.tile_pool(name="xpp", bufs=2) as xpp,
            tc.tile_pool(name="ew", bufs=2) as ew,
            tc.tile_pool(name="hp", bufs=4) as hp,
            tc.tile_pool(name="gps", bufs=2, space="PSUM") as gps,
            tc.tile_pool(name="psg", bufs=4, space="PSUM") as psg,
        ):
            def load(p, shape, dt=f16, eng=None):
                t = wp.tile(list(shape), dt, tag=p.name + "_sb")
                (eng or nc.sync).dma_start(out=t[:], in_=p[:])
                return t

            wihf_s = load(wihf, [128, 3, G])
            wihb_s = load(wihb, [128, 3, G])
            whhf_s = load(whhf, [128, 4, G])
            whhb_s = load(whhb, [128, 4, G])
            biasf_s = load(biasf, [128, 12], f32)
            biasb_s = load(biasb, [128, 12], f32)
            nb_s = load(nbias, [1, 4, 512])
            id_s = load(id128, [128, 128])
            wLf_s = load(wLf, [128, 8, G], eng=nc.gpsimd)
            wLb_s = load(wLb, [128, 8, G], eng=nc.gpsimd)
            whhLf_s = load(whhLf, [128, 4, G], eng=nc.gpsimd)
            whhLb_s = load(whhLb, [128, 4, G], eng=nc.gpsimd)
            biasLf_s = load(biasLf, [128, 12], f32, eng=nc.gpsimd)
            biasLb_s = load(biasLb, [128, 12], f32, eng=nc.gpsimd)
            w1f_s = load(w1f, [128, 4, 128], eng=nc.gpsimd)
            w1b_s = load(w1b, [128, 4, 128], eng=nc.gpsimd)

            ones_s = wp.tile([1, BL], f16, tag="ones")
            nc.vector.memset(ones_s[:], 1.0)
            ones4_s = wp.tile([128, 4, BL], f16, tag="ones4")
            nc.vector.memset(ones4_s[:], 1.0)
            onesg_s = wp.tile([1, GBC], f16, tag="onesg")
            nc.vector.memset(onesg_s[:], 1.0)

            # ---------------- GEMM machinery (per m-chunk side slots) ------
            def gemm_block_start(src_drams, kts, tag):
                """Load moving tiles for one gemm block; returns state."""
                movs = []
                for (src, kt, cols) in src_drams:
                    t = io.tile([128, kt, GBC], f16, tag=tag + "_in")
                    nc.sync.dma_start(out=t[:], in_=src[:, :, cols])
                    movs.append((t, kt))
                return movs

            def gemm_m(movs, m, w_sb, kts, bias_sb, dst, cols):
                pw = gps.tile([128, 2 * GBC], f32, tag="g_ps")
                p = pw[:, 0:GBC]
                idx = 0
                for (mt, nk) in movs:
                    for k in range(nk):
                        nc.tensor.matmul(
                            p[:],
                            w_sb[:, idx, m * 128 : (m + 1) * 128],
                            mt[:, k, :],
                            start=(idx == 0),
                            stop=(idx == kts - 1),
                        )
                        idx += 1
                xs = io.tile([128, GBC], f16, tag="g_xs")
                nc.scalar.activation(
                    xs[:], p[:], AF.Identity, bias=bias_sb[:, m : m + 1]
                )
                nc.sync.dma_start(out=dst[:, m, cols], in_=xs[:])

            def p0_block_slots(j, src, w_sb, bias_sb, dst, tag):
                """13 thunks: load + 12 m-chunks for one L0 gemm block."""
                cols = slice(j * 2 * GBC, (j + 1) * 2 * GBC)
                st = {}

                def start():
                    t = io.tile([128, 3, 2 * GBC], f16, tag=tag + "_in")
                    nc.sync.dma_start(out=t[:], in_=src[:, :, cols])
                    st["movs"] = [(t, 3)]

                def gm(m):
                    p = gps.tile([128, 2 * GBC], f32, tag="g_ps")
                    (mt, nk) = st["movs"][0]
                    for k in range(nk):
                        nc.tensor.matmul(
                            p[:], w_sb[:, k, m * 128 : (m + 1) * 128],
                            mt[:, k, :], start=(k == 0), stop=(k == nk - 1),
                        )
                    xs = io.tile([128, 2 * GBC], f16, tag="g_xs2")
                    nc.scalar.activation(
                        xs[:], p[:], AF.Identity, bias=bias_sb[:, m : m + 1]
                    )
                    nc.sync.dma_start(out=dst[:, m, cols], in_=xs[:])

                thunks = [start]
                for m in range(12):
                    thunks.append((lambda mm: lambda: gm(mm))(m))
                return thunks

            def l1_block_slots(j, w_sb, bias_sb, dst, tag):
                cols = slice(j * GBC, (j + 1) * GBC)
                st = {}

                def start():
                    st["movs"] = gemm_block_start(
                        [(yF, 4, cols), (yB, 4, cols)], 8, tag)

                thunks = [start]
                for m in range(12):
                    thunks.append(
                        (lambda mm: lambda: gemm_m(
                            st["movs"], mm, w_sb, 8, bias_sb, dst, cols))(m)
                    )
                return thunks

            # ---------------- recurrence chain ----------------------------
            def chain_init(tag, whh_sb, xp_dram, nbi, zb_act=False,
                           rev_read=False, y_dram=None, rev_write=False,
                           pooled=None):
                h = hp.tile([128, 4, BL], f16, tag=tag + "_h")
                nc.vector.memset(h[:], 0.0)
                return dict(tag=tag, whh=whh_sb, xp=xp_dram, nbi=nbi,
                            zb_act=zb_act, rev_read=rev_read, y=y_dram,
                            rev_write=rev_write, pooled=pooled, h=h,
                            xpt=None, yb=None, n=None, zb=None, rz=None,
                            b2=None, a=None,
                            yeng=nc.gpsimd if rev_write else nc.sync)

            def chain_front(ch, t):
                blk, v = t // SBLK, t % SBLK
                tag = ch["tag"]
                if v == 0:
                    sb = (NRB - 1 - blk) if ch["rev_read"] else blk
                    xpt = xpp.tile([128, 12, RB], f16, tag=tag + "_xpt")
                    nc.sync.dma_start(
                        out=xpt[:], in_=ch["xp"][:, :, sb * RB : (sb + 1) * RB]
                    )
                    ch["xpt"] = xpt
                xpt = ch["xpt"]
                cv = (SBLK - 1 - v) if ch["rev_read"] else v
                cs = slice(cv * BL, (cv + 1) * BL)
                ps = psg.tile([128, 12, BL], f32, name=tag + "_ps",
                              tag="rc_ps")
                if t == 0:
                    # first step: h = 0, gates reduce to xp/bias terms only
                    for m in range(12):
                        out = ps[:, m, :]
                        if m < 8:
                            nc.tensor.matmul(out, id_s[:], xpt[:, m, cs],
                                             start=True, stop=True)
                        else:
                            nc.tensor.matmul(
                                out,
                                nb_s[0:1, ch["nbi"],
                                     (m - 8) * 128 : (m - 7) * 128],
                                ones_s[0:1, :], start=True, stop=True,
                            )
                else:
                    # W_hh @ h(t-1) split by linearity: the a-half's operand
                    # is ready well before the b2-half, so the PE drains the
                    # a matmuls while b2 is still being computed.
                    b2, a0 = ch["b2"], ch["a"]
                    for m in range(12):
                        out = ps[:, m, :]
                        for k in range(4):
                            nc.tensor.matmul(
                                out,
                                ch["whh"][:, k, m * 128 : (m + 1) * 128],
                                a0[:, k, :],
                                start=(k == 0), stop=False,
                            )
                        for k in range(4):
                            nc.tensor.matmul(
                                out,
                                ch["whh"][:, k, m * 128 : (m + 1) * 128],
                                b2[:, k, :],
                                start=False, stop=False,
                            )
                        if m < 8:
                            nc.tensor.matmul(
                                out, id_s[:], xpt[:, m, cs],
                                start=False, stop=True,
                            )
                        else:
                            nc.tensor.matmul(
                                out,
                                nb_s[0:1, ch["nbi"],
                                     (m - 8) * 128 : (m - 7) * 128],
                                ones_s[0:1, :], start=False, stop=True,
                            )
                # one fused sigmoid over the 8 r,z chunks
                rz = ew.tile([128, 8, BL], f16, tag=tag + "_rz")
                nc.scalar.activation(rz[:], ps[:, 0:8, :], AF.Sigmoid)
                # u = ps_n * r (ps_n already includes b_hh_n)
                u = ew.tile([128, 4, BL], f16, tag=tag + "_u")
                nc.vector.tensor_tensor(
                    out=u[:], in0=ps[:, 8:12, :], in1=rz[:, 0:4, :],
                    op=OP.mult,
                )
                tn = ew.tile([128, 4, BL], f16, tag=tag + "_tn")
                nc.vector.tensor_tensor(
                    out=tn[:], in0=u[:], in1=xpt[:, 8:12, cs], op=OP.add,
                )
                n = ew.tile([128, 4, BL], f16, tag=tag + "_n")
                nc.scalar.activation(n[:], tn[:], AF.Tanh)
                zb = ew.tile([128, 4, BL], f16, tag=tag + "_zb")
                if ch["zb_act"]:
                    nc.scalar.activation(zb[:], ps[:, 4:8, :], AF.Sigmoid,
                                         scale=-1.0)
                else:
                    nc.vector.tensor_tensor(
                        out=zb[:], in0=ones4_s[:], in1=rz[:, 4:8, :],
                        op=OP.subtract,
                    )
                ch["n"], ch["zb"], ch["rz"] = n, zb, rz

            def chain_tail(ch, t):
                blk, v = t // SBLK, t % SBLK
                tag = ch["tag"]
                h = ch["h"]
                n, zb, rz = ch["n"], ch["zb"], ch["rz"]
                a = ew.tile([128, 4, BL], f16, tag=tag + "_a")
                nc.vector.tensor_tensor(
                    out=a[:], in0=rz[:, 4:8, :], in1=h[:], op=OP.mult,
                )
                ch["a"] = a
                b2 = ew.tile([128, 4, BL], f16, tag=tag + "_b2")
                nc.vector.tensor_tensor(
                    out=b2[:], in0=zb[:], in1=n[:], op=OP.mult,
                )
                ch["b2"] = b2
                hn = hp.tile([128, 4, BL], f16, tag=tag + "_h")
                nc.vector.tensor_tensor(
                    out=hn[:], in0=a[:], in1=b2[:], op=OP.add,
                )
                if ch["pooled"] is not None:
                    nc.vector.tensor_tensor(
                        out=ch["pooled"][:], in0=ch["pooled"][:], in1=hn[:],
                        op=OP.max,
                    )
                if ch["y"] is not None:
                    wv = (SBLK - 1 - v) if ch["rev_write"] else v
                    db = (NRB - 1 - blk) if ch["rev_write"] else blk
                    c0 = db * RB + wv * BL
                    ch["yeng"].dma_start(
                        out=ch["y"][:, :, c0 : c0 + BL], in_=hn[:],
                    )
                ch["h"] = hn

            def chain_step(ch, t):
                chain_front(ch, t)
                chain_tail(ch, t)

            # ---------------- phase 1: L0 ---------------------------------
            f0 = p0_block_slots(0, eTf, wihf_s, biasf_s, xp0f, "gf")
            b0 = p0_block_slots(0, eTb, wihb_s, biasb_s, xp0b, "gb")
            for th in f0 + b0:
                th()

            side1 = []
            for j in range(1, NGB // 2):
                side1 += p0_block_slots(j, eTf, wihf_s, biasf_s, xp0f, "gf")
                side1 += p0_block_slots(j, eTb, wihb_s, biasb_s, xp0b, "gb")
            side_at = {}
            for i, th in enumerate(side1):          # 1 slot per step
                side_at.setdefault(8 + i, []).append(th)

            chF = chain_init("cF", whhf_s, xp0f, 0, zb_act=True, y_dram=yF)
            chB = chain_init("cB", whhb_s, xp0b, 1, y_dram=yB,
                             rev_write=True)
            for t in range(T):
                for th in side_at.get(t, ()):
                    th()
                chain_front(chF, t)
                if t > 0:
                    chain_tail(chB, t - 1)
                chain_front(chB, t)
                chain_tail(chF, t)
            chain_tail(chB, T - 1)

            # ---------------- gap + phase 2: L1 ---------------------------
            Lf0 = l1_block_slots(0, wLf_s, biasLf_s, xpLf, "gLf")
            Lb7 = l1_block_slots(NGB - 1, wLb_s, biasLb_s, xpLb, "gLb")
            for th in Lf0 + Lb7:
                th()

            side2 = []
            for i in range(1, NGB):
                side2 += l1_block_slots(i, wLf_s, biasLf_s, xpLf, "gLf")
                side2 += l1_block_slots(NGB - 1 - i, wLb_s, biasLb_s, xpLb,
                                        "gLb")
            side_at2 = {}
            for i, th in enumerate(side2):          # 1 slot per step
                side_at2.setdefault(2 + i, []).append(th)

            pooled_f = wp.tile([128, 4, BL], f16, tag="pooled_f")
            nc.vector.memset(pooled_f[:], -60000.0)
            pooled_b = wp.tile([128, 4, BL], f16, tag="pooled_b")
            nc.vector.memset(pooled_b[:], -60000.0)

            chLf = chain_init("cLf", whhLf_s, xpLf, 2, zb_act=True, pooled=pooled_f)
            chLb = chain_init("cLb", whhLb_s, xpLb, 3, rev_read=True,
                              pooled=pooled_b)
            for t in range(T):
                for th in side_at2.get(t, ()):
                    th()
                chain_front(chLf, t)
                if t > 0:
                    chain_tail(chLb, t - 1)
                chain_front(chLb, t)
                chain_tail(chLf, t)
            chain_tail(chLb, T - 1)

            # ---------------- head: W1 @ [pooled_f; pooled_b] -------------
            hd = gps.tile([128, BL], f32, tag="head_ps", bufs=1)
            for k in range(4):
                nc.tensor.matmul(
                    hd[:], w1f_s[:, k, :], pooled_f[:, k, :],
                    start=(k == 0), stop=False,
                )
            for k in range(4):
                nc.tensor.matmul(
                    hd[:], w1b_s[:, k, :], pooled_b[:, k, :],
                    start=False, stop=(k == 3),
                )
            ho = io.tile([128, BL], f32, tag="head_sb")
            nc.vector.tensor_copy(out=ho[:], in_=hd[:])
            nc.gpsimd.dma_start(out=headout[:], in_=ho[:])

    _split_multiwaits(nc, mybir)
    try:
        ents = getattr(tc, "_perfetto_entries", None)
        span = None
        if ents:
            starts = [e[1] for e in ents if e[1] is not None]
            ends = [e[2] if e[2] is not None else e[1] for e in ents]
            if starts and ends:
                span = int(max(ends) - min(starts))
        _CACHE["model_ns"] = span
    except Exception:
        _CACHE["model_ns"] = None
    return nc


def _prep_core_inputs(inputs, c):
    """Host-side sharding/layout prep for core c (seqs 8c..8c+8)."""
    f16 = np.float16
    x = np.asarray(inputs["x"]).astype(np.int64)
    emb = np.asarray(inputs["emb"], dtype=np.float32)
    embp = np.zeros((V, EP), dtype=np.float32)
    embp[:, :E] = emb

    xg = x[c * BL : (c + 1) * BL]                     # [8, 256]
    e = embp[xg]                                      # [8, 256, 384]
    # eT[:, t*BL+b] = e[b, t]  -> [384, 2048]
    eT_f = np.ascontiguousarray(e.transpose(2, 1, 0).reshape(EP, NTOK))
    er = e[:, ::-1, :]
    eT_r = np.ascontiguousarray(er.transpose(2, 1, 0).reshape(EP, NTOK))

    def ktile(wT, kt):   # [K, G'] -> [128, kt, G']
        Kd, Gd = wT.shape
        assert Kd == kt * 128
        return np.ascontiguousarray(
            wT.reshape(kt, 128, Gd).transpose(1, 0, 2)
        ).astype(f16)

    def e3(eT):          # [384, NTOK] -> [128, 3, NTOK]
        return np.ascontiguousarray(
            eT.reshape(3, 128, NTOK).transpose(1, 0, 2)
        ).astype(f16)

    def biasrows(b_ih, b_hh):
        bv = b_ih.copy()
        bv[: 2 * H] += b_hh[: 2 * H]                  # r,z get both biases
        return np.ascontiguousarray(bv.reshape(1, 12, 128)).astype(f16)

    def biascols(b_ih, b_hh):
        bv = b_ih.copy()
        bv[: 2 * H] += b_hh[: 2 * H]
        return np.ascontiguousarray(bv.reshape(12, 128).T).astype(np.float32)

    w_ih0 = np.asarray(inputs["w_ih0"], dtype=np.float32)
    w_hh0 = np.asarray(inputs["w_hh0"], dtype=np.float32)
    b_ih0 = np.asarray(inputs["b_ih0"], dtype=np.float32)
    b_hh0 = np.asarray(inputs["b_hh0"], dtype=np.float32)
    w_ih1 = np.asarray(inputs["w_ih1"], dtype=np.float32)
    w_hh1 = np.asarray(inputs["w_hh1"], dtype=np.float32)
    b_ih1 = np.asarray(inputs["b_ih1"], dtype=np.float32)
    b_hh1 = np.asarray(inputs["b_hh1"], dtype=np.float32)
    w1 = np.asarray(inputs["w1"], dtype=np.float32)

    def wihT(d):
        w = np.zeros((G, EP), dtype=np.float32)
        w[:, :E] = w_ih0[d]
        return ktile(w.T, 3)

    nb = np.stack([
        b_hh0[0][2 * H :], b_hh0[1][2 * H :],
        b_hh1[0][2 * H :], b_hh1[1][2 * H :],
    ])[None].astype(f16)                              # [1, 4, 512]

    m = {
        "eTf": e3(eT_f),
        "eTb": e3(eT_r),
        "wihf": wihT(0),
        "wihb": wihT(1),
        "whhf": ktile(w_hh0[0].T, 4),
        "whhb": ktile(w_hh0[1].T, 4),
        "wLf": ktile(w_ih1[0].T, 8),
        "wLb": ktile(w_ih1[1].T, 8),
        "whhLf": ktile(w_hh1[0].T, 4),
        "whhLb": ktile(w_hh1[1].T, 4),
        "biasf": biascols(b_ih0[0], b_hh0[0]),
        "biasb": biascols(b_ih0[1], b_hh0[1]),
        "biasLf": biascols(b_ih1[0], b_hh1[0]),
        "biasLb": biascols(b_ih1[1], b_hh1[1]),
        "nbias": nb,
        "id128": np.eye(128, dtype=f16),
        "w1f": ktile(w1[:, :H].T, 4),
        "w1b": ktile(w1[:, H:].T, 4),
    }
    return m


def kernel(**inputs) -> np.ndarray:
    from concourse.bass_utils import run_bass_kernel_spmd

    if "nc" not in _CACHE:
        _CACHE["nc"] = _build_nc()
    nc = _CACHE["nc"]

    core_ids = list(range(8))
    in_maps = [_prep_core_inputs(inputs, c) for c in core_ids]

    res = run_bass_kernel_spmd(nc, in_maps, core_ids)
    _CACHE["last_res"] = res

    b1 = np.asarray(inputs["b1"], dtype=np.float32)
    w2 = np.asarray(inputs["w2"], dtype=np.float32)
    b2 = np.asarray(inputs["b2"], dtype=np.float32)
    out = np.zeros((B, 2), dtype=np.float32)
    for c in range(8):
        p = res.results[c]["headout"].astype(np.float32)   # [128 hid, 8]
        hid = np.maximum(p + b1[:, None], 0.0)
        logits = w2 @ hid + b2[:, None]                    # [2, 8]
        out[c * BL : (c + 1) * BL] = logits.T
    return out


# revision 5
# speedup vs baseline: 1.0143x; 1.0143x over previous
"""Bass/Trainium2 kernel for nn_GRUClassifier: 2-layer BiGRU + max-pool + MLP head.

Sharding: 8 cores x 8 sequences each, no duplicated compute, no cross-core
exchange. Each core runs two independent 8-lane recurrence chains per phase
(fwd + bwd direction of its own sequences), interleaved so each chain's
serial step latency hides behind the other's. Phase 1 = L0 both directions;
phase 2 = L1 both directions with on-the-fly max pool; W1-half head GEMM on
device; host applies relu + the 2x128 W2 classifier.

Critical-path tricks: xp contributions for the r,z gates are accumulated
into PSUM by an identity matmul; the b_hh n-gate bias comes in via a K=1
ones-row matmul; the W_hh @ h matmul is split by linearity into W@a + W@b2
(h = a + b2) so the PE drains the a-half while b2 = (1-z)*tanh-gate is still
being computed; one fused sigmoid covers all 8 r,z chunks. Per-step y
outputs go straight to DRAM from the idle SP/GpSimd DMA queues. Input
projections run as GEMM side-slots interleaved into the recurrences.

All matmul operands fp16, accumulation fp32 in PSUM. Sequence reversal for
backward chains is host-side for L0 inputs (reversed token stream),
compile-time reversed write offsets for the L0 bwd outputs, and compile-time
reversed read offsets for the L1 bwd xp stream.
"""
import os
import sys
import numpy as np

sys.path.insert(0, "/opt/trn_rl_repo")

B, T, E, H, V = 64, 256, 300, 512, 50000
EP = 384            # E padded to 3*128
G = 3 * H           # 1536 gate rows = 12 chunks of 128
BL = 8              # batch per core
NTOK = T * BL       # 2048
SBLK = 16           # steps per xpt/y block
RB = SBLK * BL      # 128 cols per recurrence block
NRB = T // SBLK     # 16 recurrence blocks
GBC = 256           # gemm block cols
NGB = NTOK // GBC   # 8 gemm blocks
GSTEP = GBC // BL   # 32 steps per gemm block

_CACHE = {}


def _patch_drain():
    """walrus CoreV3 rejects CTRL (Drain) instructions with too many sem
    waits; split the tail-drain's waits across preceding sync nops."""
    from concourse import mybir
    from concourse.tile import TileContext
    from concourse.vector_clock import ScopedClock

    if getattr(TileContext, "_drain_patched", False):
        return
    MAXW = 1

    def _drain_and_barrier(self, tick_clock, wait_clock):
        drain_inst = self.nc.sync.drain()
        wait_clock.add_sem_waits(
            drain_inst.ins, ScopedClock({None: tick_clock.global_clock})
        )
        si = drain_inst.ins.sync_info
        if si is not None and si.on_wait and len(si.on_wait) > MAXW:
            waits = list(si.on_wait)
            si.on_wait = waits[:MAXW]
            for i in range(MAXW, len(waits), MAXW):
                nop = self.nc.sync.nop(nofuse=True, hint="drain_wait_split")
                nsi = nop.ins.sync_info
                if nsi is None:
                    nop.ins.sync_info = mybir.SyncInfo(
                        on_wait=waits[i : i + MAXW], on_update=[]
                    )
                else:
                    nsi.on_wait = waits[i : i + MAXW]
        self.nc.all_engine_barrier()
        assert self.sems is not None
        popped = self.nc._tile_sem_poison_stack.pop()
        assert popped is self._sem_poison
        self.nc.clear_and_free_semaphores(list(self.sems.allocated().values()))
        self.nc.all_engine_barrier()

    TileContext._drain_and_barrier = _drain_and_barrier
    TileContext._drain_patched = True


def _split_multiwaits(nc, mybir, maxw=1):
    """walrus CoreV2/V3 setupSyncWait rejects instructions with more than one
    sem wait; split extras onto preceding same-engine nops."""
    cnt = 0
    for fn in nc.m.functions:
        for bb in fn.blocks:
            insts = bb.instructions
            out = []
            changed = False
            for inst in insts:
                si = getattr(inst, "sync_info", None)
                eng = getattr(inst, "engine", None)
                if (
                    si is not None
                    and si.on_wait
                    and len(si.on_wait) > maxw
                    and eng is not None
                    and eng != mybir.EngineType.Unassigned
                ):
                    waits = list(si.on_wait)
                    for w in waits[:-maxw]:
                        nop = mybir.InstNoOp(
                            name=f"ws_nop_{cnt}", ins=[], outs=[]
                        )
                        cnt += 1
                        nop.engine = eng
                        nop.sync_info = mybir.SyncInfo(
                            on_wait=[w], on_update=[]
                        )
                        out.append(nop)
                    si.on_wait = waits[-maxw:]
                    changed = True
                out.append(inst)
            if changed:
                bb.instructions = out
    return cnt


def _build_nc():
    from concourse import bass, mybir
    from concourse.tile import TileContext

    _patch_drain()
    f16 = mybir.dt.float16
    f32 = mybir.dt.float32
    AF = mybir.ActivationFunctionType
    OP = mybir.AluOpType

    nc = bass.Bass(target_bir_lowering=False)

    def par(name, shape, dt=f16, out=False):
        return nc.declare_dram_parameter(name, list(shape), dt, isOutput=out)

    eTf = par("eTf", [128, 3, NTOK])          # fwd-order embedded input
    eTb = par("eTb", [128, 3, NTOK])          # reversed-order
    wihf = par("wihf", [128, 3, G])           # L0 W_ih^T k-tiles
    wihb = par("wihb", [128, 3, G])
    whhf = par("whhf", [128, 4, G])           # L0 W_hh^T k-tiles
    whhb = par("whhb", [128, 4, G])
    wLf = par("wLf", [128, 8, G])             # L1 W_ih^T k-tiles (yF then yB)
    wLb = par("wLb", [128, 8, G])
    whhLf = par("whhLf", [128, 4, G])
    whhLb = par("whhLb", [128, 4, G])
    biasf = par("biasf", [128, 12], f32)      # xp bias per gate chunk col
    biasb = par("biasb", [128, 12], f32)
    biasLf = par("biasLf", [128, 12], f32)
    biasLb = par("biasLb", [128, 12], f32)
    nbias = par("nbias", [1, 4, 512])         # b_hh n-gate rows: L0f,L0b,L1f,L1b
    id128 = par("id128", [128, 128])
    w1f = par("w1f", [128, 4, 128])           # classifier W1^T k-tiles, f half
    w1b = par("w1b", [128, 4, 128])
    headout = par("headout", [128, BL], f32, out=True)

    xp0f = nc.dram_tensor("xp0f", [128, 12, NTOK], f16)
    xp0b = nc.dram_tensor("xp0b", [128, 12, NTOK], f16)
    xpLf = nc.dram_tensor("xpLf", [128, 12, NTOK], f16)
    xpLb = nc.dram_tensor("xpLb", [128, 12, NTOK], f16)
    yF = nc.dram_tensor("yF", [128, 4, NTOK], f16)
    yB = nc.dram_tensor("yB", [128, 4, NTOK], f16)

    with TileContext(nc) as tc:
        with (
            tc.tile_pool(name="wpool", bufs=1) as wp,
            tc.tile_pool(name="io", bufs=3) as io,
            tc.tile_pool(name="xpp", bufs=2) as xpp,
            tc.tile_pool(name="ew", bufs=2) as ew,
            tc.tile_pool(name="hp", bufs=4) as hp,
            tc.tile_pool(name="gps", bufs=3, space="PSUM") as gps,
            tc.tile_pool(name="psg", bufs=4, space="PSUM") as psg,
        ):
            def load(p, shape, dt=f16, eng=None):
                t = wp.tile(list(shape), dt, tag=p.name + "_sb")
                (eng or nc.sync).dma_start(out=t[:], in_=p[:])
                return t

            wihf_s = load(wihf, [128, 3, G], eng=nc.gpsimd)
            wihb_s = load(wihb, [128, 3, G], eng=nc.gpsimd)
            whhf_s = load(whhf, [128, 4, G], eng=nc.gpsimd)
            whhb_s = load(whhb, [128, 4, G], eng=nc.gpsimd)
            biasf_s = load(biasf, [128, 12], f32, eng=nc.gpsimd)
            biasb_s = load(biasb, [128, 12], f32, eng=nc.gpsimd)
            nb_s = load(nbias, [1, 4, 512], eng=nc.gpsimd)
            id_s = load(id128, [128, 128], eng=nc.gpsimd)
            wLf_s = load(wLf, [128, 8, G], eng=nc.gpsimd)
            wLb_s = load(wLb, [128, 8, G], eng=nc.gpsimd)
            whhLf_s = load(whhLf, [128, 4, G], eng=nc.gpsimd)
            whhLb_s = load(whhLb, [128, 4, G], eng=nc.gpsimd)
            biasLf_s = load(biasLf, [128, 12], f32, eng=nc.gpsimd)
            biasLb_s = load(biasLb, [128, 12], f32, eng=nc.gpsimd)
            w1f_s = load(w1f, [128, 4, 128], eng=nc.gpsimd)
            w1b_s = load(w1b, [128, 4, 128], eng=nc.gpsimd)

            ones_s = wp.tile([1, BL], f16, tag="ones")
            nc.vector.memset(ones_s[:], 1.0)
            ones4_s = wp.tile([128, 4, BL], f16, tag="ones4")
            nc.vector.memset(ones4_s[:], 1.0)
            onesg_s = wp.tile([1, GBC], f16, tag="onesg")
            nc.vector.memset(onesg_s[:], 1.0)

            # ---------------- GEMM machinery (per m-chunk side slots) ------
            def gemm_block_start(src_drams, kts, tag):
                """Load moving tiles for one gemm block; returns state."""
                movs = []
                for (src, kt, cols) in src_drams:
                    t = io.tile([128, kt, GBC], f16, tag=tag + "_in")
                    nc.sync.dma_start(out=t[:], in_=src[:, :, cols])
                    movs.append((t, kt))
                return movs

            def gemm_m(movs, m, w_sb, kts, bias_sb, dst, cols):
                pw = gps.tile([128, 2 * GBC], f32, tag="g_ps")
                p = pw[:, 0:GBC]
                idx = 0
                for (mt, nk) in movs:
                    for k in range(nk):
                        nc.tensor.matmul(
                            p[:],
                            w_sb[:, idx, m * 128 : (m + 1) * 128],
                            mt[:, k, :],
                            start=(idx == 0),
                            stop=(idx == kts - 1),
                        )
                        idx += 1
                xs = io.tile([128, GBC], f16, tag="g_xs")
                nc.scalar.activation(
                    xs[:], p[:], AF.Identity, bias=bias_sb[:, m : m + 1]
                )
                nc.sync.dma_start(out=dst[:, m, cols], in_=xs[:])

            def p0_block_slots(j, src, w_sb, bias_sb, dst, tag):
                """13 thunks: load + 12 m-chunks for one L0 gemm block."""
                cols = slice(j * 2 * GBC, (j + 1) * 2 * GBC)
                st = {}

                def start():
                    t = io.tile([128, 3, 2 * GBC], f16, tag=tag + "_in")
                    nc.sync.dma_start(out=t[:], in_=src[:, :, cols])
                    st["movs"] = [(t, 3)]

                def gm(m):
                    p = gps.tile([128, 2 * GBC], f32, tag="g_ps")
                    (mt, nk) = st["movs"][0]
                    for k in range(nk):
                        nc.tensor.matmul(
                            p[:], w_sb[:, k, m * 128 : (m + 1) * 128],
                            mt[:, k, :], start=(k == 0), stop=(k == nk - 1),
                        )
                    xs = io.tile([128, 2 * GBC], f16, tag="g_xs2")
                    nc.scalar.activation(
                        xs[:], p[:], AF.Identity, bias=bias_sb[:, m : m + 1]
                    )
                    nc.sync.dma_start(out=dst[:, m, cols], in_=xs[:])

                thunks = [start]
                for m in range(12):
                    thunks.append((lambda mm: lambda: gm(mm))(m))
                return thunks

            def l1_block_slots(j, w_sb, bias_sb, dst, tag):
                cols = slice(j * GBC, (j + 1) * GBC)
                st = {}

                def start():
                    st["movs"] = gemm_block_start(
                        [(yF, 4, cols), (yB, 4, cols)], 8, tag)

                thunks = [start]
                for m in range(12):
                    thunks.append(
                        (lambda mm: lambda: gemm_m(
                            st["movs"], mm, w_sb, 8, bias_sb, dst, cols))(m)
                    )
                return thunks

            # ---------------- recurrence chain ----------------------------
            def chain_init(tag, whh_sb, xp_dram, nbi, zb_act=False,
                           rev_read=False, y_dram=None, rev_write=False,
                           pooled=None):
                h = hp.tile([128, 4, BL], f16, tag=tag + "_h")
                nc.vector.memset(h[:], 0.0)
                return dict(tag=tag, whh=whh_sb, xp=xp_dram, nbi=nbi,
                            zb_act=zb_act, rev_read=rev_read, y=y_dram,
                            rev_write=rev_write, pooled=pooled, h=h,
                            xpt=None, yb=None, n=None, zb=None, rz=None,
                            b2=None, a=None,
                            yeng=nc.gpsimd if rev_write else nc.sync)

            def chain_front(ch, t):
                blk, v = t // SBLK, t % SBLK
                tag = ch["tag"]
                if v == 0:
                    sb = (NRB - 1 - blk) if ch["rev_read"] else blk
                    xpt = xpp.tile([128, 12, RB], f16, tag=tag + "_xpt")
                    nc.sync.dma_start(
                        out=xpt[:], in_=ch["xp"][:, :, sb * RB : (sb + 1) * RB]
                    )
                    ch["xpt"] = xpt
                xpt = ch["xpt"]
                cv = (SBLK - 1 - v) if ch["rev_read"] else v
                cs = slice(cv * BL, (cv + 1) * BL)
                ps = psg.tile([128, 12, BL], f32, name=tag + "_ps",
                              tag="rc_ps")
                if t == 0:
                    # first step: h = 0, gates reduce to xp/bias terms only
                    for m in range(12):
                        out = ps[:, m, :]
                        if m < 8:
                            nc.tensor.matmul(out, id_s[:], xpt[:, m, cs],
                                             start=True, stop=True)
                        else:
                            nc.tensor.matmul(
                                out,
                                nb_s[0:1, ch["nbi"],
                                     (m - 8) * 128 : (m - 7) * 128],
                                ones_s[0:1, :], start=True, stop=True,
                            )
                else:
                    # W_hh @ h(t-1) split by linearity: the a-half's operand
                    # is ready well before the b2-half, so the PE drains the
                    # a matmuls while b2 is still being computed.
                    b2, a0 = ch["b2"], ch["a"]
                    for m in range(12):
                        out = ps[:, m, :]
                        for k in range(4):
                            nc.tensor.matmul(
                                out,
                                ch["whh"][:, k, m * 128 : (m + 1) * 128],
                                a0[:, k, :],
                                start=(k == 0), stop=False,
                            )
                        for k in range(4):
                            nc.tensor.matmul(
                                out,
                                ch["whh"][:, k, m * 128 : (m + 1) * 128],
                                b2[:, k, :],
                                start=False, stop=False,
                            )
                        if m < 8:
                            nc.tensor.matmul(
                                out, id_s[:], xpt[:, m, cs],
                                start=False, stop=True,
                            )
                        else:
                            nc.tensor.matmul(
                                out,
                                nb_s[0:1, ch["nbi"],
                                     (m - 8) * 128 : (m - 7) * 128],
                                ones_s[0:1, :], start=False, stop=True,
                            )
                # one fused sigmoid over the 8 r,z chunks
                rz = ew.tile([128, 8, BL], f16, tag=tag + "_rz")
                nc.scalar.activation(rz[:], ps[:, 0:8, :], AF.Sigmoid)
                # u = ps_n * r (ps_n already includes b_hh_n)
                u = ew.tile([128, 4, BL], f16, tag=tag + "_u")
                nc.vector.tensor_tensor(
                    out=u[:], in0=ps[:, 8:12, :], in1=rz[:, 0:4, :],
                    op=OP.mult,
                )
                tn = ew.tile([128, 4, BL], f16, tag=tag + "_tn")
                nc.vector.tensor_tensor(
                    out=tn[:], in0=u[:], in1=xpt[:, 8:12, cs], op=OP.add,
                )
                n = ew.tile([128, 4, BL], f16, tag=tag + "_n")
                nc.scalar.activation(n[:], tn[:], AF.Tanh)
                zb = ew.tile([128, 4, BL], f16, tag=tag + "_zb")
                if ch["zb_act"]:
                    nc.scalar.activation(zb[:], ps[:, 4:8, :], AF.Sigmoid,
                                         scale=-1.0)
                else:
                    nc.vector.tensor_tensor(
                        out=zb[:], in0=ones4_s[:], in1=rz[:, 4:8, :],
                        op=OP.subtract,
                    )
                ch["n"], ch["zb"], ch["rz"] = n, zb, rz

            def chain_tail(ch, t):
                blk, v = t // SBLK, t % SBLK
                tag = ch["tag"]
                h = ch["h"]
                n, zb, rz = ch["n"], ch["zb"], ch["rz"]
                a = ew.tile([128, 4, BL], f16, tag=tag + "_a")
                nc.vector.tensor_tensor(
                    out=a[:], in0=rz[:, 4:8, :], in1=h[:], op=OP.mult,
                )
                ch["a"] = a
                b2 = ew.tile([128, 4, BL], f16, tag=tag + "_b2")
                nc.vector.tensor_tensor(
                    out=b2[:], in0=zb[:], in1=n[:], op=OP.mult,
                )
                ch["b2"] = b2
                hn = hp.tile([128, 4, BL], f16, tag=tag + "_h")
                nc.vector.tensor_tensor(
                    out=hn[:], in0=a[:], in1=b2[:], op=OP.add,
                )
                if ch["pooled"] is not None:
                    nc.vector.tensor_tensor(
                        out=ch["pooled"][:], in0=ch["pooled"][:], in1=hn[:],
                        op=OP.max,
                    )
                if ch["y"] is not None:
                    wv = (SBLK - 1 - v) if ch["rev_write"] else v
                    db = (NRB - 1 - blk) if ch["rev_write"] else blk
                    c0 = db * RB + wv * BL
                    ch["yeng"].dma_start(
                        out=ch["y"][:, :, c0 : c0 + BL], in_=hn[:],
                    )
                ch["h"] = hn

            def chain_step(ch, t):
                chain_front(ch, t)
                chain_tail(ch, t)

            # ---------------- phase 1: L0 ---------------------------------
            f0 = p0_block_slots(0, eTf, wihf_s, biasf_s, xp0f, "gf")
            b0 = p0_block_slots(0, eTb, wihb_s, biasb_s, xp0b, "gb")
            for th in f0 + b0:
                th()

            side1 = []
            for j in range(1, NGB // 2):
                side1 += p0_block_slots(j, eTf, wihf_s, biasf_s, xp0f, "gf")
                side1 += p0_block_slots(j, eTb, wihb_s, biasb_s, xp0b, "gb")
            side_at = {}
            for i, th in enumerate(side1):          # 1 slot per step
                side_at.setdefault(8 + i, []).append(th)

            chF = chain_init("cF", whhf_s, xp0f, 0, y_dram=yF)
            chB = chain_init("cB", whhb_s, xp0b, 1, y_dram=yB,
                             rev_write=True)
            for t in range(T):
                for th in side_at.get(t, ()):
                    th()
                chain_front(chF, t)
                if t > 0:
                    chain_tail(chB, t - 1)
                chain_front(chB, t)
                chain_tail(chF, t)
            chain_tail(chB, T - 1)

            # ---------------- gap + phase 2: L1 ---------------------------
            Lf0 = l1_block_slots(0, wLf_s, biasLf_s, xpLf, "gLf")
            Lb7 = l1_block_slots(NGB - 1, wLb_s, biasLb_s, xpLb, "gLb")
            for th in Lf0 + Lb7:
                th()

            side2 = []
            for i in range(1, NGB):
                side2 += l1_block_slots(i, wLf_s, biasLf_s, xpLf, "gLf")
                side2 += l1_block_slots(NGB - 1 - i, wLb_s, biasLb_s, xpLb,
                                        "gLb")
            side_at2 = {}
            for i, th in enumerate(side2):          # 1 slot per step
                side_at2.setdefault(2 + i, []).append(th)

            pooled_f = wp.tile([128, 4, BL], f16, tag="pooled_f")
            nc.vector.memset(pooled_f[:], -60000.0)
            pooled_b = wp.tile([128, 4, BL], f16, tag="pooled_b")
            nc.vector.memset(pooled_b[:], -60000.0)

            chLf = chain_init("cLf", whhLf_s, xpLf, 2, pooled=pooled_f)
            chLb = chain_init("cLb", whhLb_s, xpLb, 3, rev_read=True,
                              pooled=pooled_b)
            for t in range(T):
                for th in side_at2.get(t, ()):
                    th()
                chain_front(chLf, t)
                if t > 0:
                    chain_tail(chLb, t - 1)
                chain_front(chLb, t)
                chain_tail(chLf, t)
            chain_tail(chLb, T - 1)

            # ---------------- head: W1 @ [pooled_f; pooled_b] -------------
            hd = gps.tile([128, BL], f32, tag="head_ps", bufs=1)
            for k in range(4):
                nc.tensor.matmul(
                    hd[:], w1f_s[:, k, :], pooled_f[:, k, :],
                    start=(k == 0), stop=False,
                )
            for k in range(4):
                nc.tensor.matmul(
                    hd[:], w1b_s[:, k, :], pooled_b[:, k, :],
                    start=False, stop=(k == 3),
                )
            ho = io.tile([128, BL], f32, tag="head_sb")
            nc.vector.tensor_copy(out=ho[:], in_=hd[:])
            nc.gpsimd.dma_start(out=headout[:], in_=ho[:])

    _split_multiwaits(nc, mybir)
    try:
        ents = getattr(tc, "_perfetto_entries", None)
        span = None
        if ents:
            starts = [e[1] for e in ents if e[1] is not None]
            ends = [e[2] if e[2] is not None else e[1] for e in ents]
            if starts and ends:
                span = int(max(ends) - min(starts))
        _CACHE["model_ns"] = span
    except Exception:
        _CACHE["model_ns"] = None
    return nc


def _prep_core_inputs(inputs, c):
    """Host-side sharding/layout prep for core c (seqs 8c..8c+8)."""
    f16 = np.float16
    x = np.asarray(inputs["x"]).astype(np.int64)
    emb = np.asarray(inputs["emb"], dtype=np.float32)
    embp = np.zeros((V, EP), dtype=np.float32)
    embp[:, :E] = emb

    xg = x[c * BL : (c + 1) * BL]                     # [8, 256]
    e = embp[xg]                                      # [8, 256, 384]
    # eT[:, t*BL+b] = e[b, t]  -> [384, 2048]
    eT_f = np.ascontiguousarray(e.transpose(2, 1, 0).reshape(EP, NTOK))
    er = e[:, ::-1, :]
    eT_r = np.ascontiguousarray(er.transpose(2, 1, 0).reshape(EP, NTOK))

    def ktile(wT, kt):   # [K, G'] -> [128, kt, G']
        Kd, Gd = wT.shape
        assert Kd == kt * 128
        return np.ascontiguousarray(
            wT.reshape(kt, 128, Gd).transpose(1, 0, 2)
        ).astype(f16)

    def e3(eT):          # [384, NTOK] -> [128, 3, NTOK]
        return np.ascontiguousarray(
            eT.reshape(3, 128, NTOK).transpose(1, 0, 2)
        ).astype(f16)

    def biasrows(b_ih, b_hh):
        bv = b_ih.copy()
        bv[: 2 * H] += b_hh[: 2 * H]                  # r,z get both biases
        return np.ascontiguousarray(bv.reshape(1, 12, 128)).astype(f16)

    def biascols(b_ih, b_hh):
        bv = b_ih.copy()
        bv[: 2 * H] += b_hh[: 2 * H]
        return np.ascontiguousarray(bv.reshape(12, 128).T).astype(np.float32)

    w_ih0 = np.asarray(inputs["w_ih0"], dtype=np.float32)
    w_hh0 = np.asarray(inputs["w_hh0"], dtype=np.float32)
    b_ih0 = np.asarray(inputs["b_ih0"], dtype=np.float32)
    b_hh0 = np.asarray(inputs["b_hh0"], dtype=np.float32)
    w_ih1 = np.asarray(inputs["w_ih1"], dtype=np.float32)
    w_hh1 = np.asarray(inputs["w_hh1"], dtype=np.float32)
    b_ih1 = np.asarray(inputs["b_ih1"], dtype=np.float32)
    b_hh1 = np.asarray(inputs["b_hh1"], dtype=np.float32)
    w1 = np.asarray(inputs["w1"], dtype=np.float32)

    def wihT(d):
        w = np.zeros((G, EP), dtype=np.float32)
        w[:, :E] = w_ih0[d]
        return ktile(w.T, 3)

    nb = np.stack([
        b_hh0[0][2 * H :], b_hh0[1][2 * H :],
        b_hh1[0][2 * H :], b_hh1[1][2 * H :],
    ])[None].astype(f16)                              # [1, 4, 512]

    m = {
        "eTf": e3(eT_f),
        "eTb": e3(eT_r),
        "wihf": wihT(0),
        "wihb": wihT(1),
        "whhf": ktile(w_hh0[0].T, 4),
        "whhb": ktile(w_hh0[1].T, 4),
        "wLf": ktile(w_ih1[0].T, 8),
        "wLb": ktile(w_ih1[1].T, 8),
        "whhLf": ktile(w_hh1[0].T, 4),
        "whhLb": ktile(w_hh1[1].T, 4),
        "biasf": biascols(b_ih0[0], b_hh0[0]),
        "biasb": biascols(b_ih0[1], b_hh0[1]),
        "biasLf": biascols(b_ih1[0], b_hh1[0]),
        "biasLb": biascols(b_ih1[1], b_hh1[1]),
        "nbias": nb,
        "id128": np.eye(128, dtype=f16),
        "w1f": ktile(w1[:, :H].T, 4),
        "w1b": ktile(w1[:, H:].T, 4),
    }
    return m


def kernel(**inputs) -> np.ndarray:
    from concourse.bass_utils import run_bass_kernel_spmd

    if "nc" not in _CACHE:
        _CACHE["nc"] = _build_nc()
    nc = _CACHE["nc"]

    core_ids = list(range(8))
    in_maps = [_prep_core_inputs(inputs, c) for c in core_ids]

    res = run_bass_kernel_spmd(nc, in_maps, core_ids)
    _CACHE["last_res"] = res

    b1 = np.asarray(inputs["b1"], dtype=np.float32)
    w2 = np.asarray(inputs["w2"], dtype=np.float32)
    b2 = np.asarray(inputs["b2"], dtype=np.float32)
    out = np.zeros((B, 2), dtype=np.float32)
    for c in range(8):
        p = res.results[c]["headout"].astype(np.float32)   # [128 hid, 8]
        hid = np.maximum(p + b1[:, None], 0.0)
        logits = w2 @ hid + b2[:, None]                    # [2, 8]
        out[c * BL : (c + 1) * BL] = logits.T
    return out


# revision 6
# speedup vs baseline: 1.0153x; 1.0010x over previous
"""Bass/Trainium2 kernel for nn_GRUClassifier: 2-layer BiGRU + max-pool + MLP head.

v2 sharding: 8 cores x 8 sequences, no duplicated compute, no cross-core
exchange. Each core runs two independent 8-lane recurrence chains per phase
(fwd + bwd direction), interleaved so each chain's serial step latency hides
behind the other's. Phase 1 = L0 both directions; phase 2 = L1 both
directions with on-the-fly max pool; W1-half head GEMM on device.

Latency cuts vs v1: xp contributions for the r,z gates are accumulated into
PSUM by an identity-matmul (removes 2 DVE ops/step), the b_hh n-gate bias is
added via a K=1 ones-row matmul (removes the 4-way u-loop), and bulk DMAs are
issued from the GpSimd queue (SP sequencer was 2.4us/DMA).

All matmul operands fp16, accumulation fp32 in PSUM. Backward-direction
sequence reversal is host-side for L0 inputs (reversed token stream), output
position alignment for the L0 bwd chain is compile-time reversed write
offsets, and the L1 bwd chain reads its (position-ordered) xp stream at
compile-time reversed offsets.
"""
import os
import sys
import numpy as np

sys.path.insert(0, "/opt/trn_rl_repo")

B, T, E, H, V = 64, 256, 300, 512, 50000
EP = 384            # E padded to 3*128
G = 3 * H           # 1536 gate rows = 12 chunks of 128
BL = 8              # batch per core
NTOK = T * BL       # 2048
SBLK = 16           # steps per xpt/y block
RB = SBLK * BL      # 128 cols per recurrence block
NRB = T // SBLK     # 16 recurrence blocks
GBC = 256           # gemm block cols
NGB = NTOK // GBC   # 8 gemm blocks
GSTEP = GBC // BL   # 32 steps per gemm block

_CACHE = {}


def _patch_drain():
    """walrus CoreV3 rejects CTRL (Drain) instructions with too many sem
    waits; split the tail-drain's waits across preceding sync nops."""
    from concourse import mybir
    from concourse.tile import TileContext
    from concourse.vector_clock import ScopedClock

    if getattr(TileContext, "_drain_patched", False):
        return
    MAXW = 1

    def _drain_and_barrier(self, tick_clock, wait_clock):
        drain_inst = self.nc.sync.drain()
        wait_clock.add_sem_waits(
            drain_inst.ins, ScopedClock({None: tick_clock.global_clock})
        )
        si = drain_inst.ins.sync_info
        if si is not None and si.on_wait and len(si.on_wait) > MAXW:
            waits = list(si.on_wait)
            si.on_wait = waits[:MAXW]
            for i in range(MAXW, len(waits), MAXW):
                nop = self.nc.sync.nop(nofuse=True, hint="drain_wait_split")
                nsi = nop.ins.sync_info
                if nsi is None:
                    nop.ins.sync_info = mybir.SyncInfo(
                        on_wait=waits[i : i + MAXW], on_update=[]
                    )
                else:
                    nsi.on_wait = waits[i : i + MAXW]
        self.nc.all_engine_barrier()
        assert self.sems is not None
        popped = self.nc._tile_sem_poison_stack.pop()
        assert popped is self._sem_poison
        self.nc.clear_and_free_semaphores(list(self.sems.allocated().values()))
        self.nc.all_engine_barrier()

    TileContext._drain_and_barrier = _drain_and_barrier
    TileContext._drain_patched = True


def _split_multiwaits(nc, mybir, maxw=1):
    """walrus CoreV2/V3 setupSyncWait rejects instructions with more than one
    sem wait; split extras onto preceding same-engine nops."""
    cnt = 0
    for fn in nc.m.functions:
        for bb in fn.blocks:
            insts = bb.instructions
            out = []
            changed = False
            for inst in insts:
                si = getattr(inst, "sync_info", None)
                eng = getattr(inst, "engine", None)
                if (
                    si is not None
                    and si.on_wait
                    and len(si.on_wait) > maxw
                    and eng is not None
                    and eng != mybir.EngineType.Unassigned
                ):
                    waits = list(si.on_wait)
                    for w in waits[:-maxw]:
                        nop = mybir.InstNoOp(
                            name=f"ws_nop_{cnt}", ins=[], outs=[]
                        )
                        cnt += 1
                        nop.engine = eng
                        nop.sync_info = mybir.SyncInfo(
                            on_wait=[w], on_update=[]
                        )
                        out.append(nop)
                    si.on_wait = waits[-maxw:]
                    changed = True
                out.append(inst)
            if changed:
                bb.instructions = out
    return cnt


def _build_nc():
    from concourse import bass, mybir
    from concourse.tile import TileContext

    _patch_drain()
    f16 = mybir.dt.float16
    f32 = mybir.dt.float32
    AF = mybir.ActivationFunctionType
    OP = mybir.AluOpType

    nc = bass.Bass(target_bir_lowering=False)

    def par(name, shape, dt=f16, out=False):
        return nc.declare_dram_parameter(name, list(shape), dt, isOutput=out)

    eTf = par("eTf", [128, 3, NTOK])          # fwd-order embedded input
    eTb = par("eTb", [128, 3, NTOK])          # reversed-order
    wihf = par("wihf", [128, 3, G])           # L0 W_ih^T k-tiles
    wihb = par("wihb", [128, 3, G])
    whhf = par("whhf", [128, 4, G])           # L0 W_hh^T k-tiles
    whhb = par("whhb", [128, 4, G])
    wLf = par("wLf", [128, 8, G])             # L1 W_ih^T k-tiles (yF then yB)
    wLb = par("wLb", [128, 8, G])
    whhLf = par("whhLf", [128, 4, G])
    whhLb = par("whhLb", [128, 4, G])
    biasf = par("biasf", [128, 12], f32)      # xp bias per gate chunk col
    biasb = par("biasb", [128, 12], f32)
    biasLf = par("biasLf", [128, 12], f32)
    biasLb = par("biasLb", [128, 12], f32)
    nbias = par("nbias", [1, 4, 512])         # b_hh n-gate rows: L0f,L0b,L1f,L1b
    id128 = par("id128", [128, 128])
    w1f = par("w1f", [128, 4, 128])           # classifier W1^T k-tiles, f half
    w1b = par("w1b", [128, 4, 128])
    headout = par("headout", [128, BL], f32, out=True)

    xp0f = nc.dram_tensor("xp0f", [128, 12, NTOK], f16)
    xp0b = nc.dram_tensor("xp0b", [128, 12, NTOK], f16)
    xpLf = nc.dram_tensor("xpLf", [128, 12, NTOK], f16)
    xpLb = nc.dram_tensor("xpLb", [128, 12, NTOK], f16)
    yF = nc.dram_tensor("yF", [128, 4, NTOK], f16)
    yB = nc.dram_tensor("yB", [128, 4, NTOK], f16)

    with TileContext(nc) as tc:
        with (
            tc.tile_pool(name="wpool", bufs=1) as wp,
            tc.tile_pool(name="io", bufs=3) as io,
            tc.tile_pool(name="xpp", bufs=2) as xpp,
            tc.tile_pool(name="ew", bufs=2) as ew,
            tc.tile_pool(name="hp", bufs=4) as hp,
            tc.tile_pool(name="gps", bufs=3, space="PSUM") as gps,
            tc.tile_pool(name="psg", bufs=4, space="PSUM") as psg,
        ):
            def load(p, shape, dt=f16, eng=None):
                t = wp.tile(list(shape), dt, tag=p.name + "_sb")
                (eng or nc.sync).dma_start(out=t[:], in_=p[:])
                return t

            wihf_s = load(wihf, [128, 3, G], eng=nc.gpsimd)
            wihb_s = load(wihb, [128, 3, G], eng=nc.gpsimd)
            whhf_s = load(whhf, [128, 4, G], eng=nc.gpsimd)
            whhb_s = load(whhb, [128, 4, G], eng=nc.gpsimd)
            biasf_s = load(biasf, [128, 12], f32, eng=nc.gpsimd)
            biasb_s = load(biasb, [128, 12], f32, eng=nc.gpsimd)
            nb_s = load(nbias, [1, 4, 512], eng=nc.gpsimd)
            id_s = load(id128, [128, 128], eng=nc.gpsimd)
            wLf_s = load(wLf, [128, 8, G], eng=nc.gpsimd)
            wLb_s = load(wLb, [128, 8, G], eng=nc.gpsimd)
            whhLf_s = load(whhLf, [128, 4, G], eng=nc.gpsimd)
            whhLb_s = load(whhLb, [128, 4, G], eng=nc.gpsimd)
            biasLf_s = load(biasLf, [128, 12], f32, eng=nc.gpsimd)
            biasLb_s = load(biasLb, [128, 12], f32, eng=nc.gpsimd)
            w1f_s = load(w1f, [128, 4, 128], eng=nc.gpsimd)
            w1b_s = load(w1b, [128, 4, 128], eng=nc.gpsimd)

            ones_s = wp.tile([1, BL], f16, tag="ones")
            nc.vector.memset(ones_s[:], 1.0)
            ones4_s = wp.tile([128, 4, BL], f16, tag="ones4")
            nc.vector.memset(ones4_s[:], 1.0)
            onesg_s = wp.tile([1, GBC], f16, tag="onesg")
            nc.vector.memset(onesg_s[:], 1.0)

            # ---------------- GEMM machinery (per m-chunk side slots) ------
            def gemm_block_start(src_drams, kts, tag):
                """Load moving tiles for one gemm block; returns state."""
                movs = []
                for (src, kt, cols) in src_drams:
                    t = io.tile([128, kt, GBC], f16, tag=tag + "_in")
                    nc.sync.dma_start(out=t[:], in_=src[:, :, cols])
                    movs.append((t, kt))
                return movs

            def epilogue(xs, p, bias_sb, m, alt):
                if alt and m % 2 == 1:
                    nc.vector.tensor_scalar(
                        out=xs, in0=p, scalar1=bias_sb[:, m : m + 1],
                        scalar2=None, op0=OP.add,
                    )
                else:
                    nc.scalar.activation(
                        xs, p, AF.Identity, bias=bias_sb[:, m : m + 1]
                    )

            def gemm_m(movs, m, w_sb, kts, bias_sb, dst, cols, alt=False):
                pw = gps.tile([128, 2 * GBC], f32, tag="g_ps")
                p = pw[:, 0:GBC]
                idx = 0
                for (mt, nk) in movs:
                    for k in range(nk):
                        nc.tensor.matmul(
                            p[:],
                            w_sb[:, idx, m * 128 : (m + 1) * 128],
                            mt[:, k, :],
                            start=(idx == 0),
                            stop=(idx == kts - 1),
                        )
                        idx += 1
                xs = io.tile([128, GBC], f16, tag="g_xs")
                epilogue(xs[:], p[:], bias_sb, m, alt)
                nc.sync.dma_start(out=dst[:, m, cols], in_=xs[:])

            def p0_block_slots(j, src, w_sb, bias_sb, dst, tag,
                               alt=False):
                """13 thunks: load + 12 m-chunks for one L0 gemm block."""
                cols = slice(j * 2 * GBC, (j + 1) * 2 * GBC)
                st = {}

                def start():
                    t = io.tile([128, 3, 2 * GBC], f16, tag=tag + "_in")
                    nc.sync.dma_start(out=t[:], in_=src[:, :, cols])
                    st["movs"] = [(t, 3)]

                def gm(m):
                    p = gps.tile([128, 2 * GBC], f32, tag="g_ps")
                    (mt, nk) = st["movs"][0]
                    for k in range(nk):
                        nc.tensor.matmul(
                            p[:], w_sb[:, k, m * 128 : (m + 1) * 128],
                            mt[:, k, :], start=(k == 0), stop=(k == nk - 1),
                        )
                    xs = io.tile([128, 2 * GBC], f16, tag="g_xs2")
                    epilogue(xs[:], p[:], bias_sb, m, alt)
                    nc.sync.dma_start(out=dst[:, m, cols], in_=xs[:])

                thunks = [start]
                for m in range(12):
                    thunks.append((lambda mm: lambda: gm(mm))(m))
                return thunks

            def l1_block_slots(j, w_sb, bias_sb, dst, tag, alt=False):
                cols = slice(j * GBC, (j + 1) * GBC)
                st = {}

                def start():
                    st["movs"] = gemm_block_start(
                        [(yF, 4, cols), (yB, 4, cols)], 8, tag)

                thunks = [start]
                for m in range(12):
                    thunks.append(
                        (lambda mm: lambda: gemm_m(
                            st["movs"], mm, w_sb, 8, bias_sb, dst, cols,
                            alt=alt))(m)
                    )
                return thunks

            # ---------------- recurrence chain ----------------------------
            def chain_init(tag, whh_sb, xp_dram, nbi, zb_act=False,
                           rev_read=False, y_dram=None, rev_write=False,
                           pooled=None):
                h = hp.tile([128, 4, BL], f16, tag=tag + "_h")
                nc.vector.memset(h[:], 0.0)
                return dict(tag=tag, whh=whh_sb, xp=xp_dram, nbi=nbi,
                            zb_act=zb_act, rev_read=rev_read, y=y_dram,
                            rev_write=rev_write, pooled=pooled, h=h,
                            xpt=None, yb=None, n=None, zb=None, rz=None,
                            b2=None, a=None,
                            yeng=nc.gpsimd if rev_write else nc.sync)

            def chain_front(ch, t):
                blk, v = t // SBLK, t % SBLK
                tag = ch["tag"]
                if v == 0:
                    sb = (NRB - 1 - blk) if ch["rev_read"] else blk
                    xpt = xpp.tile([128, 12, RB], f16, tag=tag + "_xpt")
                    nc.sync.dma_start(
                        out=xpt[:], in_=ch["xp"][:, :, sb * RB : (sb + 1) * RB]
                    )
                    ch["xpt"] = xpt
                xpt = ch["xpt"]
                cv = (SBLK - 1 - v) if ch["rev_read"] else v
                cs = slice(cv * BL, (cv + 1) * BL)
                ps = psg.tile([128, 12, BL], f32, name=tag + "_ps",
                              tag="rc_ps")
                if t == 0:
                    # first step: h = 0, gates reduce to xp/bias terms only
                    for m in range(12):
                        out = ps[:, m, :]
                        if m < 8:
                            nc.tensor.matmul(out, id_s[:], xpt[:, m, cs],
                                             start=True, stop=True)
                        else:
                            nc.tensor.matmul(
                                out,
                                nb_s[0:1, ch["nbi"],
                                     (m - 8) * 128 : (m - 7) * 128],
                                ones_s[0:1, :], start=True, stop=True,
                            )
                else:
                    # W_hh @ h(t-1) split by linearity: the a-half's operand
                    # is ready well before the b2-half, so the PE drains the
                    # a matmuls while b2 is still being computed.
                    b2, a0 = ch["b2"], ch["a"]
                    for m in range(12):
                        out = ps[:, m, :]
                        for k in range(4):
                            nc.tensor.matmul(
                                out,
                                ch["whh"][:, k, m * 128 : (m + 1) * 128],
                                a0[:, k, :],
                                start=(k == 0), stop=False,
                            )
                        for k in range(4):
                            nc.tensor.matmul(
                                out,
                                ch["whh"][:, k, m * 128 : (m + 1) * 128],
                                b2[:, k, :],
                                start=False, stop=False,
                            )
                        if m < 8:
                            nc.tensor.matmul(
                                out, id_s[:], xpt[:, m, cs],
                                start=False, stop=True,
                            )
                        else:
                            nc.tensor.matmul(
                                out,
                                nb_s[0:1, ch["nbi"],
                                     (m - 8) * 128 : (m - 7) * 128],
                                ones_s[0:1, :], start=False, stop=True,
                            )
                # one fused sigmoid over the 8 r,z chunks
                rz = ew.tile([128, 8, BL], f16, tag=tag + "_rz")
                nc.scalar.activation(rz[:], ps[:, 0:8, :], AF.Sigmoid)
                # u = ps_n * r (ps_n already includes b_hh_n)
                u = ew.tile([128, 4, BL], f16, tag=tag + "_u")
                nc.vector.tensor_tensor(
                    out=u[:], in0=ps[:, 8:12, :], in1=rz[:, 0:4, :],
                    op=OP.mult,
                )
                tn = ew.tile([128, 4, BL], f16, tag=tag + "_tn")
                nc.vector.tensor_tensor(
                    out=tn[:], in0=u[:], in1=xpt[:, 8:12, cs], op=OP.add,
                )
                n = ew.tile([128, 4, BL], f16, tag=tag + "_n")
                nc.scalar.activation(n[:], tn[:], AF.Tanh)
                zb = ew.tile([128, 4, BL], f16, tag=tag + "_zb")
                if ch["zb_act"]:
                    nc.scalar.activation(zb[:], ps[:, 4:8, :], AF.Sigmoid,
                                         scale=-1.0)
                else:
                    nc.vector.tensor_tensor(
                        out=zb[:], in0=ones4_s[:], in1=rz[:, 4:8, :],
                        op=OP.subtract,
                    )
                ch["n"], ch["zb"], ch["rz"] = n, zb, rz

            def chain_tail(ch, t):
                blk, v = t // SBLK, t % SBLK
                tag = ch["tag"]
                h = ch["h"]
                n, zb, rz = ch["n"], ch["zb"], ch["rz"]
                a = ew.tile([128, 4, BL], f16, tag=tag + "_a")
                nc.vector.tensor_tensor(
                    out=a[:], in0=rz[:, 4:8, :], in1=h[:], op=OP.mult,
                )
                ch["a"] = a
                b2 = ew.tile([128, 4, BL], f16, tag=tag + "_b2")
                nc.vector.tensor_tensor(
                    out=b2[:], in0=zb[:], in1=n[:], op=OP.mult,
                )
                ch["b2"] = b2
                hn = hp.tile([128, 4, BL], f16, tag=tag + "_h")
                nc.vector.tensor_tensor(
                    out=hn[:], in0=a[:], in1=b2[:], op=OP.add,
                )
                if ch["pooled"] is not None:
                    nc.vector.tensor_tensor(
                        out=ch["pooled"][:], in0=ch["pooled"][:], in1=hn[:],
                        op=OP.max,
                    )
                if ch["y"] is not None:
                    wv = (SBLK - 1 - v) if ch["rev_write"] else v
                    db = (NRB - 1 - blk) if ch["rev_write"] else blk
                    c0 = db * RB + wv * BL
                    ch["yeng"].dma_start(
                        out=ch["y"][:, :, c0 : c0 + BL], in_=hn[:],
                    )
                ch["h"] = hn

            def chain_step(ch, t):
                chain_front(ch, t)
                chain_tail(ch, t)

            # ---------------- phase 1: L0 ---------------------------------
            f0 = p0_block_slots(0, eTf, wihf_s, biasf_s, xp0f, "gf",
                                alt=True)
            b0 = p0_block_slots(0, eTb, wihb_s, biasb_s, xp0b, "gb",
                                alt=True)
            for th in f0 + b0:
                th()

            side1 = []
            for j in range(1, NGB // 2):
                side1 += p0_block_slots(j, eTf, wihf_s, biasf_s, xp0f, "gf")
                side1 += p0_block_slots(j, eTb, wihb_s, biasb_s, xp0b, "gb")
            side_at = {}
            for i, th in enumerate(side1):          # 1 slot per step
                side_at.setdefault(8 + i, []).append(th)

            chF = chain_init("cF", whhf_s, xp0f, 0, y_dram=yF)
            chB = chain_init("cB", whhb_s, xp0b, 1, y_dram=yB,
                             rev_write=True)
            for t in range(T):
                for th in side_at.get(t, ()):
                    th()
                chain_front(chF, t)
                if t > 0:
                    chain_tail(chB, t - 1)
                chain_front(chB, t)
                chain_tail(chF, t)
            chain_tail(chB, T - 1)

            # ---------------- gap + phase 2: L1 ---------------------------
            Lf0 = l1_block_slots(0, wLf_s, biasLf_s, xpLf, "gLf",
                                 alt=True)
            Lb7 = l1_block_slots(NGB - 1, wLb_s, biasLb_s, xpLb, "gLb",
                                 alt=True)
            for th in Lf0 + Lb7:
                th()

            side2 = []
            for i in range(1, NGB):
                side2 += l1_block_slots(i, wLf_s, biasLf_s, xpLf, "gLf")
                side2 += l1_block_slots(NGB - 1 - i, wLb_s, biasLb_s, xpLb,
                                        "gLb")
            side_at2 = {}
            for i, th in enumerate(side2):          # 1 slot per step
                side_at2.setdefault(2 + i, []).append(th)

            pooled_f = wp.tile([128, 4, BL], f16, tag="pooled_f")
            nc.vector.memset(pooled_f[:], -60000.0)
            pooled_b = wp.tile([128, 4, BL], f16, tag="pooled_b")
            nc.vector.memset(pooled_b[:], -60000.0)

            chLf = chain_init("cLf", whhLf_s, xpLf, 2, pooled=pooled_f)
            chLb = chain_init("cLb", whhLb_s, xpLb, 3, rev_read=True,
                              pooled=pooled_b)
            for t in range(T):
                for th in side_at2.get(t, ()):
                    th()
                chain_front(chLf, t)
                if t > 0:
                    chain_tail(chLb, t - 1)
                chain_front(chLb, t)
                chain_tail(chLf, t)
            chain_tail(chLb, T - 1)

            # ---------------- head: W1 @ [pooled_f; pooled_b] -------------
            hd = gps.tile([128, BL], f32, tag="head_ps", bufs=1)
            for k in range(4):
                nc.tensor.matmul(
                    hd[:], w1f_s[:, k, :], pooled_f[:, k, :],
                    start=(k == 0), stop=False,
                )
            for k in range(4):
                nc.tensor.matmul(
                    hd[:], w1b_s[:, k, :], pooled_b[:, k, :],
                    start=False, stop=(k == 3),
                )
            ho = io.tile([128, BL], f32, tag="head_sb")
            nc.vector.tensor_copy(out=ho[:], in_=hd[:])
            nc.gpsimd.dma_start(out=headout[:], in_=ho[:])

    _split_multiwaits(nc, mybir)
    try:
        ents = getattr(tc, "_perfetto_entries", None)
        span = None
        if ents:
            starts = [e[1] for e in ents if e[1] is not None]
            ends = [e[2] if e[2] is not None else e[1] for e in ents]
            if starts and ends:
                span = int(max(ends) - min(starts))
        _CACHE["model_ns"] = span
    except Exception:
        _CACHE["model_ns"] = None
    return nc


def _prep_core_inputs(inputs, c):
    """Host-side sharding/layout prep for core c (seqs 8c..8c+8)."""
    f16 = np.float16
    x = np.asarray(inputs["x"]).astype(np.int64)
    emb = np.asarray(inputs["emb"], dtype=np.float32)
    embp = np.zeros((V, EP), dtype=np.float32)
    embp[:, :E] = emb

    xg = x[c * BL : (c + 1) * BL]                     # [8, 256]
    e = embp[xg]                                      # [8, 256, 384]
    # eT[:, t*BL+b] = e[b, t]  -> [384, 2048]
    eT_f = np.ascontiguousarray(e.transpose(2, 1, 0).reshape(EP, NTOK))
    er = e[:, ::-1, :]
    eT_r = np.ascontiguousarray(er.transpose(2, 1, 0).reshape(EP, NTOK))

    def ktile(wT, kt):   # [K, G'] -> [128, kt, G']
        Kd, Gd = wT.shape
        assert Kd == kt * 128
        return np.ascontiguousarray(
            wT.reshape(kt, 128, Gd).transpose(1, 0, 2)
        ).astype(f16)

    def e3(eT):          # [384, NTOK] -> [128, 3, NTOK]
        return np.ascontiguousarray(
            eT.reshape(3, 128, NTOK).transpose(1, 0, 2)
        ).astype(f16)

    def biasrows(b_ih, b_hh):
        bv = b_ih.copy()
        bv[: 2 * H] += b_hh[: 2 * H]                  # r,z get both biases
        return np.ascontiguousarray(bv.reshape(1, 12, 128)).astype(f16)

    def biascols(b_ih, b_hh):
        bv = b_ih.copy()
        bv[: 2 * H] += b_hh[: 2 * H]
        return np.ascontiguousarray(bv.reshape(12, 128).T).astype(np.float32)

    w_ih0 = np.asarray(inputs["w_ih0"], dtype=np.float32)
    w_hh0 = np.asarray(inputs["w_hh0"], dtype=np.float32)
    b_ih0 = np.asarray(inputs["b_ih0"], dtype=np.float32)
    b_hh0 = np.asarray(inputs["b_hh0"], dtype=np.float32)
    w_ih1 = np.asarray(inputs["w_ih1"], dtype=np.float32)
    w_hh1 = np.asarray(inputs["w_hh1"], dtype=np.float32)
    b_ih1 = np.asarray(inputs["b_ih1"], dtype=np.float32)
    b_hh1 = np.asarray(inputs["b_hh1"], dtype=np.float32)
    w1 = np.asarray(inputs["w1"], dtype=np.float32)

    def wihT(d):
        w = np.zeros((G, EP), dtype=np.float32)
        w[:, :E] = w_ih0[d]
        return ktile(w.T, 3)

    nb = np.stack([
        b_hh0[0][2 * H :], b_hh0[1][2 * H :],
        b_hh1[0][2 * H :], b_hh1[1][2 * H :],
    ])[None].astype(f16)                              # [1, 4, 512]

    m = {
        "eTf": e3(eT_f),
        "eTb": e3(eT_r),
        "wihf": wihT(0),
        "wihb": wihT(1),
        "whhf": ktile(w_hh0[0].T, 4),
        "whhb": ktile(w_hh0[1].T, 4),
        "wLf": ktile(w_ih1[0].T, 8),
        "wLb": ktile(w_ih1[1].T, 8),
        "whhLf": ktile(w_hh1[0].T, 4),
        "whhLb": ktile(w_hh1[1].T, 4),
        "biasf": biascols(b_ih0[0], b_hh0[0]),
        "biasb": biascols(b_ih0[1], b_hh0[1]),
        "biasLf": biascols(b_ih1[0], b_hh1[0]),
        "biasLb": biascols(b_ih1[1], b_hh1[1]),
        "nbias": nb,
        "id128": np.eye(128, dtype=f16),
        "w1f": ktile(w1[:, :H].T, 4),
        "w1b": ktile(w1[:, H:].T, 4),
    }
    return m


def kernel(**inputs) -> np.ndarray:
    from concourse.bass_utils import run_bass_kernel_spmd

    if "nc" not in _CACHE:
        _CACHE["nc"] = _build_nc()
    nc = _CACHE["nc"]

    core_ids = list(range(8))
    in_maps = [_prep_core_inputs(inputs, c) for c in core_ids]

    res = run_bass_kernel_spmd(nc, in_maps, core_ids)
    _CACHE["last_res"] = res

    b1 = np.asarray(inputs["b1"], dtype=np.float32)
    w2 = np.asarray(inputs["w2"], dtype=np.float32)
    b2 = np.asarray(inputs["b2"], dtype=np.float32)
    out = np.zeros((B, 2), dtype=np.float32)
    for c in range(8):
        p = res.results[c]["headout"].astype(np.float32)   # [128 hid, 8]
        hid = np.maximum(p + b1[:, None], 0.0)
        logits = w2 @ hid + b2[:, None]                    # [2, 8]
        out[c * BL : (c + 1) * BL] = logits.T
    return out


# revision 7
# speedup vs baseline: 1.0167x; 1.0014x over previous
"""Bass/Trainium2 kernel for nn_GRUClassifier: 2-layer BiGRU + max-pool + MLP head.

v2 sharding: 8 cores x 8 sequences, no duplicated compute, no cross-core
exchange. Each core runs two independent 8-lane recurrence chains per phase
(fwd + bwd direction), interleaved so each chain's serial step latency hides
behind the other's. Phase 1 = L0 both directions; phase 2 = L1 both
directions with on-the-fly max pool; W1-half head GEMM on device.

Latency cuts vs v1: xp contributions for the r,z gates are accumulated into
PSUM by an identity-matmul (removes 2 DVE ops/step), the b_hh n-gate bias is
added via a K=1 ones-row matmul (removes the 4-way u-loop), and bulk DMAs are
issued from the GpSimd queue (SP sequencer was 2.4us/DMA).

All matmul operands fp16, accumulation fp32 in PSUM. Backward-direction
sequence reversal is host-side for L0 inputs (reversed token stream), output
position alignment for the L0 bwd chain is compile-time reversed write
offsets, and the L1 bwd chain reads its (position-ordered) xp stream at
compile-time reversed offsets.
"""
import os
import sys
import numpy as np

sys.path.insert(0, "/opt/trn_rl_repo")

B, T, E, H, V = 64, 256, 300, 512, 50000
EP = 384            # E padded to 3*128
G = 3 * H           # 1536 gate rows = 12 chunks of 128
BL = 8              # batch per core
NTOK = T * BL       # 2048
SBLK = 16           # steps per xpt/y block
RB = SBLK * BL      # 128 cols per recurrence block
NRB = T // SBLK     # 16 recurrence blocks
GBC = 256           # gemm block cols
NGB = NTOK // GBC   # 8 gemm blocks
GSTEP = GBC // BL   # 32 steps per gemm block

_CACHE = {}


def _patch_drain():
    """walrus CoreV3 rejects CTRL (Drain) instructions with too many sem
    waits; split the tail-drain's waits across preceding sync nops."""
    from concourse import mybir
    from concourse.tile import TileContext
    from concourse.vector_clock import ScopedClock

    if getattr(TileContext, "_drain_patched", False):
        return
    MAXW = 1

    def _drain_and_barrier(self, tick_clock, wait_clock):
        drain_inst = self.nc.sync.drain()
        wait_clock.add_sem_waits(
            drain_inst.ins, ScopedClock({None: tick_clock.global_clock})
        )
        si = drain_inst.ins.sync_info
        if si is not None and si.on_wait and len(si.on_wait) > MAXW:
            waits = list(si.on_wait)
            si.on_wait = waits[:MAXW]
            for i in range(MAXW, len(waits), MAXW):
                nop = self.nc.sync.nop(nofuse=True, hint="drain_wait_split")
                nsi = nop.ins.sync_info
                if nsi is None:
                    nop.ins.sync_info = mybir.SyncInfo(
                        on_wait=waits[i : i + MAXW], on_update=[]
                    )
                else:
                    nsi.on_wait = waits[i : i + MAXW]
        self.nc.all_engine_barrier()
        assert self.sems is not None
        popped = self.nc._tile_sem_poison_stack.pop()
        assert popped is self._sem_poison
        self.nc.clear_and_free_semaphores(list(self.sems.allocated().values()))
        self.nc.all_engine_barrier()

    TileContext._drain_and_barrier = _drain_and_barrier
    TileContext._drain_patched = True


def _split_multiwaits(nc, mybir, maxw=1):
    """walrus CoreV2/V3 setupSyncWait rejects instructions with more than one
    sem wait; split extras onto preceding same-engine nops."""
    cnt = 0
    for fn in nc.m.functions:
        for bb in fn.blocks:
            insts = bb.instructions
            out = []
            changed = False
            for inst in insts:
                si = getattr(inst, "sync_info", None)
                eng = getattr(inst, "engine", None)
                if (
                    si is not None
                    and si.on_wait
                    and len(si.on_wait) > maxw
                    and eng is not None
                    and eng != mybir.EngineType.Unassigned
                ):
                    waits = list(si.on_wait)
                    for w in waits[:-maxw]:
                        nop = mybir.InstNoOp(
                            name=f"ws_nop_{cnt}", ins=[], outs=[]
                        )
                        cnt += 1
                        nop.engine = eng
                        nop.sync_info = mybir.SyncInfo(
                            on_wait=[w], on_update=[]
                        )
                        out.append(nop)
                    si.on_wait = waits[-maxw:]
                    changed = True
                out.append(inst)
            if changed:
                bb.instructions = out
    return cnt


def _build_nc():
    from concourse import bass, mybir
    from concourse.tile import TileContext

    _patch_drain()
    f16 = mybir.dt.float16
    f32 = mybir.dt.float32
    AF = mybir.ActivationFunctionType
    OP = mybir.AluOpType

    nc = bass.Bass(target_bir_lowering=False)

    def par(name, shape, dt=f16, out=False):
        return nc.declare_dram_parameter(name, list(shape), dt, isOutput=out)

    eTf = par("eTf", [128, 3, NTOK])          # fwd-order embedded input
    eTb = par("eTb", [128, 3, NTOK])          # reversed-order
    wihf = par("wihf", [128, 3, G])           # L0 W_ih^T k-tiles
    wihb = par("wihb", [128, 3, G])
    whhf = par("whhf", [128, 4, G])           # L0 W_hh^T k-tiles
    whhb = par("whhb", [128, 4, G])
    wLf = par("wLf", [128, 8, G])             # L1 W_ih^T k-tiles (yF then yB)
    wLb = par("wLb", [128, 8, G])
    whhLf = par("whhLf", [128, 4, G])
    whhLb = par("whhLb", [128, 4, G])
    biasf = par("biasf", [128, 12], f32)      # xp bias per gate chunk col
    biasb = par("biasb", [128, 12], f32)
    biasLf = par("biasLf", [128, 12], f32)
    biasLb = par("biasLb", [128, 12], f32)
    nbias = par("nbias", [1, 4, 512])         # b_hh n-gate rows: L0f,L0b,L1f,L1b
    id128 = par("id128", [128, 128])
    w1f = par("w1f", [128, 4, 128])           # classifier W1^T k-tiles, f half
    w1b = par("w1b", [128, 4, 128])
    headout = par("headout", [128, BL], f32, out=True)

    # block-major xp streams: one contiguous 3KB run per partition per
    # recurrence block, so the per-block xpt load is 128 descriptors
    xp0f = nc.dram_tensor("xp0f", [128, NRB, 12, RB], f16)
    xp0b = nc.dram_tensor("xp0b", [128, NRB, 12, RB], f16)
    xpLf = nc.dram_tensor("xpLf", [128, NRB, 12, RB], f16)
    xpLb = nc.dram_tensor("xpLb", [128, NRB, 12, RB], f16)
    yF = nc.dram_tensor("yF", [128, 4, NTOK], f16)
    yB = nc.dram_tensor("yB", [128, 4, NTOK], f16)

    with TileContext(nc) as tc:
        with (
            tc.tile_pool(name="wpool", bufs=1) as wp,
            tc.tile_pool(name="io", bufs=3) as io,
            tc.tile_pool(name="xpp", bufs=2) as xpp,
            tc.tile_pool(name="ew", bufs=2) as ew,
            tc.tile_pool(name="hp", bufs=4) as hp,
            tc.tile_pool(name="gps", bufs=3, space="PSUM") as gps,
            tc.tile_pool(name="psg", bufs=4, space="PSUM") as psg,
        ):
            def load(p, shape, dt=f16, eng=None):
                t = wp.tile(list(shape), dt, tag=p.name + "_sb")
                (eng or nc.sync).dma_start(out=t[:], in_=p[:])
                return t

            wihf_s = load(wihf, [128, 3, G], eng=nc.gpsimd)
            wihb_s = load(wihb, [128, 3, G], eng=nc.gpsimd)
            whhf_s = load(whhf, [128, 4, G], eng=nc.gpsimd)
            whhb_s = load(whhb, [128, 4, G], eng=nc.gpsimd)
            biasf_s = load(biasf, [128, 12], f32, eng=nc.gpsimd)
            biasb_s = load(biasb, [128, 12], f32, eng=nc.gpsimd)
            nb_s = load(nbias, [1, 4, 512], eng=nc.gpsimd)
            id_s = load(id128, [128, 128], eng=nc.gpsimd)
            wLf_s = load(wLf, [128, 8, G], eng=nc.gpsimd)
            wLb_s = load(wLb, [128, 8, G], eng=nc.gpsimd)
            whhLf_s = load(whhLf, [128, 4, G], eng=nc.gpsimd)
            whhLb_s = load(whhLb, [128, 4, G], eng=nc.gpsimd)
            biasLf_s = load(biasLf, [128, 12], f32, eng=nc.gpsimd)
            biasLb_s = load(biasLb, [128, 12], f32, eng=nc.gpsimd)
            w1f_s = load(w1f, [128, 4, 128], eng=nc.gpsimd)
            w1b_s = load(w1b, [128, 4, 128], eng=nc.gpsimd)

            ones_s = wp.tile([1, BL], f16, tag="ones")
            nc.vector.memset(ones_s[:], 1.0)
            ones4_s = wp.tile([128, 4, BL], f16, tag="ones4")
            nc.vector.memset(ones4_s[:], 1.0)
            onesg_s = wp.tile([1, GBC], f16, tag="onesg")
            nc.vector.memset(onesg_s[:], 1.0)

            # ---------------- GEMM machinery (per m-chunk side slots) ------
            def gemm_block_start(src_drams, kts, tag):
                """Load moving tiles for one gemm block; returns state."""
                movs = []
                for (src, kt, cols) in src_drams:
                    t = io.tile([128, kt, GBC], f16, tag=tag + "_in")
                    nc.sync.dma_start(out=t[:], in_=src[:, :, cols])
                    movs.append((t, kt))
                return movs

            def epilogue(xs, p, bias_sb, m, alt):
                if alt and m % 2 == 1:
                    nc.vector.tensor_scalar(
                        out=xs, in0=p, scalar1=bias_sb[:, m : m + 1],
                        scalar2=None, op0=OP.add,
                    )
                else:
                    nc.scalar.activation(
                        xs, p, AF.Identity, bias=bias_sb[:, m : m + 1]
                    )

            def gemm_m(movs, m, w_sb, kts, bias_sb, dst, cols, alt=False):
                pw = gps.tile([128, 2 * GBC], f32, tag="g_ps")
                p = pw[:, 0:GBC]
                idx = 0
                for (mt, nk) in movs:
                    for k in range(nk):
                        nc.tensor.matmul(
                            p[:],
                            w_sb[:, idx, m * 128 : (m + 1) * 128],
                            mt[:, k, :],
                            start=(idx == 0),
                            stop=(idx == kts - 1),
                        )
                        idx += 1
                xs = io.tile([128, GBC], f16, tag="g_xs")
                epilogue(xs[:], p[:], bias_sb, m, alt)
                nc.sync.dma_start(
                    out=dst[:, cols.start // RB : cols.stop // RB, m, :],
                    in_=xs[:],
                )

            def p0_block_slots(j, src, w_sb, bias_sb, dst, tag,
                               alt=False):
                """13 thunks: load + 12 m-chunks for one L0 gemm block."""
                cols = slice(j * 2 * GBC, (j + 1) * 2 * GBC)
                st = {}

                def start():
                    t = io.tile([128, 3, 2 * GBC], f16, tag=tag + "_in")
                    nc.sync.dma_start(out=t[:], in_=src[:, :, cols])
                    st["movs"] = [(t, 3)]

                def gm(m):
                    p = gps.tile([128, 2 * GBC], f32, tag="g_ps")
                    (mt, nk) = st["movs"][0]
                    for k in range(nk):
                        nc.tensor.matmul(
                            p[:], w_sb[:, k, m * 128 : (m + 1) * 128],
                            mt[:, k, :], start=(k == 0), stop=(k == nk - 1),
                        )
                    xs = io.tile([128, 2 * GBC], f16, tag="g_xs2")
                    epilogue(xs[:], p[:], bias_sb, m, alt)
                    nc.sync.dma_start(
                        out=dst[:, cols.start // RB : cols.stop // RB, m, :],
                        in_=xs[:],
                    )

                thunks = [start]
                for m in range(12):
                    thunks.append((lambda mm: lambda: gm(mm))(m))
                return thunks

            def l1_block_slots(j, w_sb, bias_sb, dst, tag, alt=False):
                cols = slice(j * GBC, (j + 1) * GBC)
                st = {}

                def start():
                    st["movs"] = gemm_block_start(
                        [(yF, 4, cols), (yB, 4, cols)], 8, tag)

                thunks = [start]
                for m in range(12):
                    thunks.append(
                        (lambda mm: lambda: gemm_m(
                            st["movs"], mm, w_sb, 8, bias_sb, dst, cols,
                            alt=alt))(m)
                    )
                return thunks

            # ---------------- recurrence chain ----------------------------
            def chain_init(tag, whh_sb, xp_dram, nbi, zb_act=False,
                           rev_read=False, y_dram=None, rev_write=False,
                           pooled=None):
                h = hp.tile([128, 4, BL], f16, tag=tag + "_h")
                nc.vector.memset(h[:], 0.0)
                return dict(tag=tag, whh=whh_sb, xp=xp_dram, nbi=nbi,
                            zb_act=zb_act, rev_read=rev_read, y=y_dram,
                            rev_write=rev_write, pooled=pooled, h=h,
                            xpt=None, yb=None, n=None, zb=None, rz=None,
                            b2=None, a=None,
                            yeng=nc.gpsimd if rev_write else nc.sync)

            def chain_front(ch, t):
                blk, v = t // SBLK, t % SBLK
                tag = ch["tag"]
                if v == 0:
                    sb = (NRB - 1 - blk) if ch["rev_read"] else blk
                    xpt = xpp.tile([128, 12, RB], f16, tag=tag + "_xpt")
                    nc.sync.dma_start(
                        out=xpt[:], in_=ch["xp"][:, sb, :, :]
                    )
                    ch["xpt"] = xpt
                xpt = ch["xpt"]
                cv = (SBLK - 1 - v) if ch["rev_read"] else v
                cs = slice(cv * BL, (cv + 1) * BL)
                ps = psg.tile([128, 12, BL], f32, name=tag + "_ps",
                              tag="rc_ps")
                if t == 0:
                    # first step: h = 0, gates reduce to xp/bias terms only
                    for m in range(12):
                        out = ps[:, m, :]
                        if m < 8:
                            nc.tensor.matmul(out, id_s[:], xpt[:, m, cs],
                                             start=True, stop=True)
                        else:
                            nc.tensor.matmul(
                                out,
                                nb_s[0:1, ch["nbi"],
                                     (m - 8) * 128 : (m - 7) * 128],
                                ones_s[0:1, :], start=True, stop=True,
                            )
                else:
                    # W_hh @ h(t-1) split by linearity: the a-half's operand
                    # is ready well before the b2-half, so the PE drains the
                    # a matmuls while b2 is still being computed.
                    b2, a0 = ch["b2"], ch["a"]
                    for m in range(12):
                        out = ps[:, m, :]
                        for k in range(4):
                            nc.tensor.matmul(
                                out,
                                ch["whh"][:, k, m * 128 : (m + 1) * 128],
                                a0[:, k, :],
                                start=(k == 0), stop=False,
                            )
                        for k in range(4):
                            nc.tensor.matmul(
                                out,
                                ch["whh"][:, k, m * 128 : (m + 1) * 128],
                                b2[:, k, :],
                                start=False, stop=False,
                            )
                        if m < 8:
                            nc.tensor.matmul(
                                out, id_s[:], xpt[:, m, cs],
                                start=False, stop=True,
                            )
                        else:
                            nc.tensor.matmul(
                                out,
                                nb_s[0:1, ch["nbi"],
                                     (m - 8) * 128 : (m - 7) * 128],
                                ones_s[0:1, :], start=False, stop=True,
                            )
                # one fused sigmoid over the 8 r,z chunks
                rz = ew.tile([128, 8, BL], f16, tag=tag + "_rz")
                nc.scalar.activation(rz[:], ps[:, 0:8, :], AF.Sigmoid)
                # u = ps_n * r (ps_n already includes b_hh_n)
                u = ew.tile([128, 4, BL], f16, tag=tag + "_u")
                nc.vector.tensor_tensor(
                    out=u[:], in0=ps[:, 8:12, :], in1=rz[:, 0:4, :],
                    op=OP.mult,
                )
                tn = ew.tile([128, 4, BL], f16, tag=tag + "_tn")
                nc.vector.tensor_tensor(
                    out=tn[:], in0=u[:], in1=xpt[:, 8:12, cs], op=OP.add,
                )
                n = ew.tile([128, 4, BL], f16, tag=tag + "_n")
                nc.scalar.activation(n[:], tn[:], AF.Tanh)
                zb = ew.tile([128, 4, BL], f16, tag=tag + "_zb")
                if ch["zb_act"]:
                    nc.scalar.activation(zb[:], ps[:, 4:8, :], AF.Sigmoid,
                                         scale=-1.0)
                else:
                    nc.vector.tensor_tensor(
                        out=zb[:], in0=ones4_s[:], in1=rz[:, 4:8, :],
                        op=OP.subtract,
                    )
                ch["n"], ch["zb"], ch["rz"] = n, zb, rz

            def chain_tail(ch, t):
                blk, v = t // SBLK, t % SBLK
                tag = ch["tag"]
                h = ch["h"]
                n, zb, rz = ch["n"], ch["zb"], ch["rz"]
                a = ew.tile([128, 4, BL], f16, tag=tag + "_a")
                nc.vector.tensor_tensor(
                    out=a[:], in0=rz[:, 4:8, :], in1=h[:], op=OP.mult,
                )
                ch["a"] = a
                b2 = ew.tile([128, 4, BL], f16, tag=tag + "_b2")
                nc.vector.tensor_tensor(
                    out=b2[:], in0=zb[:], in1=n[:], op=OP.mult,
                )
                ch["b2"] = b2
                hn = hp.tile([128, 4, BL], f16, tag=tag + "_h")
                nc.vector.tensor_tensor(
                    out=hn[:], in0=a[:], in1=b2[:], op=OP.add,
                )
                if ch["pooled"] is not None:
                    nc.vector.tensor_tensor(
                        out=ch["pooled"][:], in0=ch["pooled"][:], in1=hn[:],
                        op=OP.max,
                    )
                if ch["y"] is not None:
                    wv = (SBLK - 1 - v) if ch["rev_write"] else v
                    db = (NRB - 1 - blk) if ch["rev_write"] else blk
                    c0 = db * RB + wv * BL
                    ch["yeng"].dma_start(
                        out=ch["y"][:, :, c0 : c0 + BL], in_=hn[:],
                    )
                ch["h"] = hn

            def chain_step(ch, t):
                chain_front(ch, t)
                chain_tail(ch, t)

            # ---------------- phase 1: L0 ---------------------------------
            f0 = p0_block_slots(0, eTf, wihf_s, biasf_s, xp0f, "gf",
                                alt=True)
            b0 = p0_block_slots(0, eTb, wihb_s, biasb_s, xp0b, "gb",
                                alt=True)
            for th in f0 + b0:
                th()

            side1 = []
            for j in range(1, NGB // 2):
                side1 += p0_block_slots(j, eTf, wihf_s, biasf_s, xp0f, "gf")
                side1 += p0_block_slots(j, eTb, wihb_s, biasb_s, xp0b, "gb")
            side_at = {}
            for i, th in enumerate(side1):          # 1 slot per step
                side_at.setdefault(8 + i, []).append(th)

            chF = chain_init("cF", whhf_s, xp0f, 0, y_dram=yF)
            chB = chain_init("cB", whhb_s, xp0b, 1, y_dram=yB,
                             rev_write=True)
            for t in range(T):
                for th in side_at.get(t, ()):
                    th()
                chain_front(chF, t)
                if t > 0:
                    chain_tail(chB, t - 1)
                chain_front(chB, t)
                chain_tail(chF, t)
            chain_tail(chB, T - 1)

            # ---------------- gap + phase 2: L1 ---------------------------
            Lf0 = l1_block_slots(0, wLf_s, biasLf_s, xpLf, "gLf",
                                 alt=True)
            Lb7 = l1_block_slots(NGB - 1, wLb_s, biasLb_s, xpLb, "gLb",
                                 alt=True)
            for th in Lf0 + Lb7:
                th()

            side2 = []
            for i in range(1, NGB):
                side2 += l1_block_slots(i, wLf_s, biasLf_s, xpLf, "gLf")
                side2 += l1_block_slots(NGB - 1 - i, wLb_s, biasLb_s, xpLb,
                                        "gLb")
            side_at2 = {}
            for i, th in enumerate(side2):          # 1 slot per step
                side_at2.setdefault(2 + i, []).append(th)

            pooled_f = wp.tile([128, 4, BL], f16, tag="pooled_f")
            nc.vector.memset(pooled_f[:], -60000.0)
            pooled_b = wp.tile([128, 4, BL], f16, tag="pooled_b")
            nc.vector.memset(pooled_b[:], -60000.0)

            chLf = chain_init("cLf", whhLf_s, xpLf, 2, pooled=pooled_f)
            chLb = chain_init("cLb", whhLb_s, xpLb, 3, rev_read=True,
                              pooled=pooled_b)
            for t in range(T):
                for th in side_at2.get(t, ()):
                    th()
                chain_front(chLf, t)
                if t > 0:
                    chain_tail(chLb, t - 1)
                chain_front(chLb, t)
                chain_tail(chLf, t)
            chain_tail(chLb, T - 1)

            # ---------------- head: W1 @ [pooled_f; pooled_b] -------------
            hd = gps.tile([128, BL], f32, tag="head_ps", bufs=1)
            for k in range(4):
                nc.tensor.matmul(
                    hd[:], w1f_s[:, k, :], pooled_f[:, k, :],
                    start=(k == 0), stop=False,
                )
            for k in range(4):
                nc.tensor.matmul(
                    hd[:], w1b_s[:, k, :], pooled_b[:, k, :],
                    start=False, stop=(k == 3),
                )
            ho = io.tile([128, BL], f32, tag="head_sb")
            nc.vector.tensor_copy(out=ho[:], in_=hd[:])
            nc.gpsimd.dma_start(out=headout[:], in_=ho[:])

    _split_multiwaits(nc, mybir)
    try:
        ents = getattr(tc, "_perfetto_entries", None)
        span = None
        if ents:
            starts = [e[1] for e in ents if e[1] is not None]
            ends = [e[2] if e[2] is not None else e[1] for e in ents]
            if starts and ends:
                span = int(max(ends) - min(starts))
        _CACHE["model_ns"] = span
    except Exception:
        _CACHE["model_ns"] = None
    return nc


def _prep_core_inputs(inputs, c):
    """Host-side sharding/layout prep for core c (seqs 8c..8c+8)."""
    f16 = np.float16
    x = np.asarray(inputs["x"]).astype(np.int64)
    emb = np.asarray(inputs["emb"], dtype=np.float32)
    embp = np.zeros((V, EP), dtype=np.float32)
    embp[:, :E] = emb

    xg = x[c * BL : (c + 1) * BL]                     # [8, 256]
    e = embp[xg]                                      # [8, 256, 384]
    # eT[:, t*BL+b] = e[b, t]  -> [384, 2048]
    eT_f = np.ascontiguousarray(e.transpose(2, 1, 0).reshape(EP, NTOK))
    er = e[:, ::-1, :]
    eT_r = np.ascontiguousarray(er.transpose(2, 1, 0).reshape(EP, NTOK))

    def ktile(wT, kt):   # [K, G'] -> [128, kt, G']
        Kd, Gd = wT.shape
        assert Kd == kt * 128
        return np.ascontiguousarray(
            wT.reshape(kt, 128, Gd).transpose(1, 0, 2)
        ).astype(f16)

    def e3(eT):          # [384, NTOK] -> [128, 3, NTOK]
        return np.ascontiguousarray(
            eT.reshape(3, 128, NTOK).transpose(1, 0, 2)
        ).astype(f16)

    def biasrows(b_ih, b_hh):
        bv = b_ih.copy()
        bv[: 2 * H] += b_hh[: 2 * H]                  # r,z get both biases
        return np.ascontiguousarray(bv.reshape(1, 12, 128)).astype(f16)

    def biascols(b_ih, b_hh):
        bv = b_ih.copy()
        bv[: 2 * H] += b_hh[: 2 * H]
        return np.ascontiguousarray(bv.reshape(12, 128).T).astype(np.float32)

    w_ih0 = np.asarray(inputs["w_ih0"], dtype=np.float32)
    w_hh0 = np.asarray(inputs["w_hh0"], dtype=np.float32)
    b_ih0 = np.asarray(inputs["b_ih0"], dtype=np.float32)
    b_hh0 = np.asarray(inputs["b_hh0"], dtype=np.float32)
    w_ih1 = np.asarray(inputs["w_ih1"], dtype=np.float32)
    w_hh1 = np.asarray(inputs["w_hh1"], dtype=np.float32)
    b_ih1 = np.asarray(inputs["b_ih1"], dtype=np.float32)
    b_hh1 = np.asarray(inputs["b_hh1"], dtype=np.float32)
    w1 = np.asarray(inputs["w1"], dtype=np.float32)

    def wihT(d):
        w = np.zeros((G, EP), dtype=np.float32)
        w[:, :E] = w_ih0[d]
        return ktile(w.T, 3)

    nb = np.stack([
        b_hh0[0][2 * H :], b_hh0[1][2 * H :],
        b_hh1[0][2 * H :], b_hh1[1][2 * H :],
    ])[None].astype(f16)                              # [1, 4, 512]

    m = {
        "eTf": e3(eT_f),
        "eTb": e3(eT_r),
        "wihf": wihT(0),
        "wihb": wihT(1),
        "whhf": ktile(w_hh0[0].T, 4),
        "whhb": ktile(w_hh0[1].T, 4),
        "wLf": ktile(w_ih1[0].T, 8),
        "wLb": ktile(w_ih1[1].T, 8),
        "whhLf": ktile(w_hh1[0].T, 4),
        "whhLb": ktile(w_hh1[1].T, 4),
        "biasf": biascols(b_ih0[0], b_hh0[0]),
        "biasb": biascols(b_ih0[1], b_hh0[1]),
        "biasLf": biascols(b_ih1[0], b_hh1[0]),
        "biasLb": biascols(b_ih1[1], b_hh1[1]),
        "nbias": nb,
        "id128": np.eye(128, dtype=f16),
        "w1f": ktile(w1[:, :H].T, 4),
        "w1b": ktile(w1[:, H:].T, 4),
    }
    return m


def kernel(**inputs) -> np.ndarray:
    from concourse.bass_utils import run_bass_kernel_spmd

    if "nc" not in _CACHE:
        _CACHE["nc"] = _build_nc()
    nc = _CACHE["nc"]

    core_ids = list(range(8))
    in_maps = [_prep_core_inputs(inputs, c) for c in core_ids]

    res = run_bass_kernel_spmd(nc, in_maps, core_ids)
    _CACHE["last_res"] = res

    b1 = np.asarray(inputs["b1"], dtype=np.float32)
    w2 = np.asarray(inputs["w2"], dtype=np.float32)
    b2 = np.asarray(inputs["b2"], dtype=np.float32)
    out = np.zeros((B, 2), dtype=np.float32)
    for c in range(8):
        p = res.results[c]["headout"].astype(np.float32)   # [128 hid, 8]
        hid = np.maximum(p + b1[:, None], 0.0)
        logits = w2 @ hid + b2[:, None]                    # [2, 8]
        out[c * BL : (c + 1) * BL] = logits.T
    return out
